# revision 1
# baseline (speedup 1.0000x reference)
"""Trainium2 Bass kernel for nn_EngramModule: single-query top-k memory attention
with gated residual + LayerNorm, data-parallel across 8 NeuronCores.

Contract: kernel(**inputs) takes the FULL unsharded inputs and returns the FULL
(8192, 1024) float32 output.

Per-core pipeline (1024 batch rows, 8 row-tiles of 128):
  A:  Q = h @ Wq                              (TensorE, float32r)
  B:  per (tile, k): Kp/Vp projections on TensorE; scores + online softmax
      (no max-subtraction; logits are ~N(0,1)) + attn-weighted V on VectorE
  C:  memory_out = attnout @ Wo; gate = [h|mo] @ Wg (+bg on VectorE);
      sigmoid(x) = 0.5*tanh(x/2)+0.5 keeps ScalarE in one ACT table set;
      aug = h + g*mo; LayerNorm finalizes per tile with a VectorE-only
      Newton rsqrt (bit-trick init), so no second ACT table set, no spill,
      and no serial epilogue phase.

Bulk weight streams ride the ScalarE HWDGE queue; latency-critical per-tile
loads ride the SyncE queue so they are never stuck behind megabyte weight
transfers. Activations are pre-laid-out on host (pure transpose/reshape,
lossless) so the contraction dim sits on SBUF partitions and no on-chip
transposes of the big tensors are needed.
"""

import os
import sys

import numpy as np

for _p in ("/opt/trn_rl_repo", "/root/.axon_site/_ro/trn_rl_repo"):
    if os.path.isdir(_p) and _p not in sys.path:
        sys.path.insert(0, _p)

from contextlib import ExitStack

import concourse.bacc as bacc
import concourse.mybir as mybir
import concourse.tile as tile
from concourse.bass_utils import run_bass_kernel_spmd

F32 = mybir.dt.float32
F32R = mybir.dt.float32r
BF16 = mybir.dt.bfloat16
I32 = mybir.dt.int32
AX = mybir.AxisListType
OP = mybir.AluOpType
AF = mybir.ActivationFunctionType

N_CORES = 8
B = 8192
HID = 1024
NH = 16
DH = 64
TOPK = 8
LN_EPS = 1e-5

BC = B // N_CORES          # rows per core = 1024
NT = BC // 128             # row-tiles per core = 8
NIC = HID // 128           # 128-row contraction chunks = 8
NJH = HID // 512           # 512-wide output halves = 2
SCALE = DH ** -0.5
RSQRT_MAGIC = 0x5F3759DF

# Set by test.py to collect a profile; grading path leaves this off.
TRACE = False

_CACHE = {}


def _build(nt=NT):
    nc = bacc.Bacc("TRN2", target_bir_lowering=False, debug=False,
                   num_devices=N_CORES)

    # ---- DRAM parameters (per-core shard, host-prepped layouts) ----
    h_d = nc.declare_dram_parameter("h", [nt, 128, HID], F32, isOutput=False)
    mkT_d = nc.declare_dram_parameter("mkT", [nt, TOPK, 128, NIC, 128], BF16, isOutput=False)
    mvT_d = nc.declare_dram_parameter("mvT", [nt, TOPK, 128, NIC, 128], BF16, isOutput=False)
    hTb_d = nc.declare_dram_parameter("hTb", [nt, 128, NIC, 128], BF16, isOutput=False)
    wq_d = nc.declare_dram_parameter("Wq", [128, NIC, HID], BF16, isOutput=False)
    wk_d = nc.declare_dram_parameter("Wk", [128, NIC, HID], BF16, isOutput=False)
    wv_d = nc.declare_dram_parameter("Wv", [128, NIC, HID], BF16, isOutput=False)
    wo_d = nc.declare_dram_parameter("Wo", [128, NIC, HID], F32, isOutput=False)
    wg_d = nc.declare_dram_parameter("Wg", [128, 2 * NIC, HID], BF16, isOutput=False)
    bgb_d = nc.declare_dram_parameter("bgB", [128, HID], F32, isOutput=False)
    eye_d = nc.declare_dram_parameter("eye", [128, 128], F32, isOutput=False)
    lng_d = nc.declare_dram_parameter("lngB", [128, HID], F32, isOutput=False)
    lnb_d = nc.declare_dram_parameter("lnbB", [128, HID], F32, isOutput=False)
    out_d = nc.declare_dram_parameter("out", [nt, 128, HID], F32, isOutput=True)

    def r(ap):
        return ap.bitcast(F32R)

    def load_w(tile_sb, dram, nchunk, cast=r):
        # bulk weights on the ScalarE HWDGE queue, chunked so the first
        # dependent matmul only waits for its own chunk
        for ic in range(nchunk):
            nc.scalar.dma_start(tile_sb[:, ic, :], cast(dram.ap()[:, ic, :]))

    with ExitStack() as octx:
        tc = octx.enter_context(tile.TileContext(nc))

        pers = octx.enter_context(tc.tile_pool(name="pers", bufs=1))
        sum_all = pers.tile([128, nt], F32, tag="sum_all")
        ss_all = pers.tile([128, nt], F32, tag="ss_all")

        # Wo is loaded during A/B (scalar queue) and consumed in C
        pWO_cm = tc.tile_pool(name="pWO", bufs=1); pWO = pWO_cm.__enter__()
        wo_sb = pWO.tile([128, NIC, HID], F32R, tag="wo")

        # attnout stays SBUF-resident from B into C (no DRAM spill)
        pAO_cm = tc.tile_pool(name="pAO", bufs=1); pAO = pAO_cm.__enter__()
        ao_all = pAO.tile([128, nt, HID], F32, tag="ao_all")

        # q_all is the only tile whose lifetime spans two phases (A -> B)
        pAB_cm = tc.tile_pool(name="pAB", bufs=1); pAB = pAB_cm.__enter__()
        q_all = pAB.tile([128, nt, HID], F32, tag="q_all")

        # Wk/Wv stream in during phase A on the scalar queue
        pWKV_cm = tc.tile_pool(name="wkv", bufs=1); wkv = pWKV_cm.__enter__()
        wk_sb = wkv.tile([128, NIC, HID], BF16, tag="wk")
        wv_sb = wkv.tile([128, NIC, HID], BF16, tag="wv")

        # ================= phase A: Q projection =================
        with ExitStack() as actx:
            wqp = actx.enter_context(tc.tile_pool(name="wq", bufs=1))
            hp = actx.enter_context(tc.tile_pool(name="hT_a", bufs=1))
            qps = actx.enter_context(tc.tile_pool(name="q_ps", bufs=2, space="PSUM"))
            wq_sb = wqp.tile([128, NIC, HID], BF16, tag="wq")
            load_w(wq_sb, wq_d, NIC, cast=(lambda ap: ap))
            load_w(wk_sb, wk_d, NIC, cast=(lambda ap: ap))
            load_w(wv_sb, wv_d, NIC, cast=(lambda ap: ap))
            # one resident hT tile, chunk-DMAed up front: the in-order sync
            # queue never stalls on buffer recycling, so phase B's first
            # mkT/mvT prefetch lands right behind it
            hT_all = hp.tile([128, nt, NIC, 128], BF16, tag="hT")
            for t in range(nt):
                nc.sync.dma_start(hT_all[:, t], hTb_d.ap()[t])
            for t in range(nt):
                q_ps = qps.tile([128, HID], F32, tag="qps")
                for ic in range(NIC):
                    for jh in range(NJH):
                        nc.tensor.matmul(
                            q_ps[:, jh * 512:(jh + 1) * 512],
                            hT_all[:, t, ic, :],
                            wq_sb[:, ic, jh * 512:(jh + 1) * 512],
                            start=(ic == 0), stop=(ic == NIC - 1),
                        )
                nc.scalar.copy(q_all[:, t, :], q_ps[:])

        # ================= phase B: attention =================
        with ExitStack() as bctx:
            mp = bctx.enter_context(tc.tile_pool(name="mkv", bufs=4))
            load_w(wo_sb, wo_d, NIC)
            preload = {}
            for (pt, pk) in ((0, 0),):
                a = mp.tile([128, NIC, 128], BF16, tag="mkT")
                nc.sync.dma_start(a[:], mkT_d.ap()[pt, pk])
                b_ = mp.tile([128, NIC, 128], BF16, tag="mvT")
                nc.sync.dma_start(b_[:], mvT_d.ap()[pt, pk])
                preload[(pt, pk)] = (a, b_)
            kvps = bctx.enter_context(tc.tile_pool(name="kv_ps", bufs=2, space="PSUM"))
            sp = bctx.enter_context(tc.tile_pool(name="scr", bufs=2))
            accp = bctx.enter_context(tc.tile_pool(name="acc", bufs=2))
            ep = bctx.enter_context(tc.tile_pool(name="e", bufs=2))

            for t in range(nt):
                acc = accp.tile([128, HID], F32, tag="acc")
                e_all = ep.tile([128, TOPK, NH], F32, tag="e_all")
                q_t = q_all[:, t, :]
                for k in range(TOPK):
                    if (t, k) in preload:
                        mkT, mvT = preload[(t, k)]
                    else:
                        mkT = mp.tile([128, NIC, 128], BF16, tag="mkT")
                        nc.sync.dma_start(mkT[:], mkT_d.ap()[t, k])
                        mvT = mp.tile([128, NIC, 128], BF16, tag="mvT")
                        nc.sync.dma_start(mvT[:], mvT_d.ap()[t, k])

                    kp_ps = kvps.tile([128, HID], F32, tag="kp")
                    vp_ps = kvps.tile([128, HID], F32, tag="vp")
                    for ic in range(NIC):
                        for jh in range(NJH):
                            nc.tensor.matmul(
                                kp_ps[:, jh * 512:(jh + 1) * 512],
                                mkT[:, ic, :],
                                wk_sb[:, ic, jh * 512:(jh + 1) * 512],
                                start=(ic == 0), stop=(ic == NIC - 1),
                            )
                    for ic in range(NIC):
                        for jh in range(NJH):
                            nc.tensor.matmul(
                                vp_ps[:, jh * 512:(jh + 1) * 512],
                                mvT[:, ic, :],
                                wv_sb[:, ic, jh * 512:(jh + 1) * 512],
                                start=(ic == 0), stop=(ic == NIC - 1),
                            )

                    # scores for all 16 heads of this k-slot
                    p_scr = sp.tile([128, HID], F32, tag="p")
                    nc.vector.tensor_mul(p_scr[:], q_t, kp_ps[:])
                    s_k = ep.tile([128, NH], F32, tag="s_k")
                    nc.vector.reduce_sum(
                        s_k[:], p_scr[:].rearrange("p (h d) -> p h d", h=NH), axis=AX.X)
                    # e = exp(scores * DH**-0.5); logits ~N(0,1) so no max-sub
                    nc.scalar.activation(e_all[:, k, :], s_k[:], AF.Exp, scale=SCALE)

                    # weighted V accumulate: acc += e[:,k,h] (bcast over d) * Vp
                    e_bc = e_all[:, k, :].unsqueeze(2).broadcast_to([128, NH, DH])
                    dst = acc if k == 0 else sp.tile([128, HID], F32, tag="pv")
                    nc.vector.tensor_tensor(
                        dst[:].rearrange("p (h d) -> p h d", h=NH),
                        vp_ps[:].rearrange("p (h d) -> p h d", h=NH),
                        e_bc, op=OP.mult)
                    if k > 0:
                        nc.gpsimd.tensor_add(acc[:], acc[:], dst[:])

                # normalize: attnout = acc * (1/sum_k e)
                den = ep.tile([128, NH], F32, tag="den")
                nc.vector.reduce_sum(
                    den[:], e_all[:].rearrange("p k h -> p h k"), axis=AX.X)
                rden = ep.tile([128, NH], F32, tag="rden")
                nc.vector.reciprocal(rden[:], den[:])
                rden_bc = rden[:].unsqueeze(2).broadcast_to([128, NH, DH])
                nc.vector.tensor_tensor(
                    ao_all[:, t, :].rearrange("p (h d) -> p h d", h=NH),
                    acc[:].rearrange("p (h d) -> p h d", h=NH),
                    rden_bc, op=OP.mult)

        pWKV_cm.__exit__(None, None, None)  # release Wk/Wv
        pAB_cm.__exit__(None, None, None)   # release q_all

        # ===== phase C: memory_out, gate, residual, LayerNorm, output =====
        with ExitStack() as cctx:
            eyep = cctx.enter_context(tc.tile_pool(name="eye", bufs=1))
            wop = cctx.enter_context(tc.tile_pool(name="wog", bufs=1))
            cstr = cctx.enter_context(tc.tile_pool(name="c_str", bufs=2))
            csb = cctx.enter_context(tc.tile_pool(name="c_sb", bufs=2))
            stp = cctx.enter_context(tc.tile_pool(name="stats", bufs=2))
            tps = cctx.enter_context(tc.tile_pool(name="tp_ps", bufs=2, space="PSUM"))
            mps = cctx.enter_context(tc.tile_pool(name="mo_ps", bufs=1, space="PSUM"))
            gps = cctx.enter_context(tc.tile_pool(name="g_ps", bufs=1, space="PSUM"))

            # small latency-critical constants on the sync queue, first
            eye_sb = eyep.tile([128, 128], F32, tag="eye")
            nc.sync.dma_start(eye_sb[:], eye_d.ap())
            bgb_sb = eyep.tile([128, HID], F32, tag="bgb")
            nc.sync.dma_start(bgb_sb[:], bgb_d.ap())
            lng_sb = eyep.tile([128, HID], F32, tag="lng")
            nc.sync.dma_start(lng_sb[:], lng_d.ap())
            lnb_sb = eyep.tile([128, HID], F32, tag="lnb")
            nc.sync.dma_start(lnb_sb[:], lnb_d.ap())

            wg_sb = wop.tile([128, 2 * NIC, HID], BF16, tag="wg")
            load_w(wg_sb, wg_d, 2 * NIC, cast=(lambda ap: ap))

            for t in range(nt):
                hT_sb = cstr.tile([128, NIC, 128], BF16, tag="hT_c")
                nc.sync.dma_start(hT_sb[:], hTb_d.ap()[t])
                h_sb = cstr.tile([128, HID], F32, tag="h_c")
                nc.sync.dma_start(h_sb[:], h_d.ap()[t])

                at_ps = tps.tile([128, NIC, 128], F32, tag="tp_ps")
                for ic in range(NIC):
                    nc.tensor.transpose(
                        at_ps[:, ic, :], ao_all[:, t, ic * 128:(ic + 1) * 128],
                        eye_sb[:])
                atT_sb = csb.tile([128, NIC, 128], F32R, tag="atT")
                nc.scalar.copy(atT_sb[:], at_ps[:])

                mo_ps = mps.tile([128, HID], F32, tag="mo_ps")
                for ic in range(NIC):
                    for jh in range(NJH):
                        nc.tensor.matmul(
                            mo_ps[:, jh * 512:(jh + 1) * 512],
                            atT_sb[:, ic, :],
                            wo_sb[:, ic, jh * 512:(jh + 1) * 512],
                            start=(ic == 0), stop=(ic == NIC - 1),
                        )
                mo_sb = csb.tile([128, HID], F32, tag="mo")
                nc.scalar.copy(mo_sb[:], mo_ps[:])

                # gate h-part first: fills TensorE while mo copies out of PSUM
                g_ps = gps.tile([128, HID], F32, tag="g_ps")
                for ic in range(NIC):
                    for jh in range(NJH):
                        sl = slice(jh * 512, (jh + 1) * 512)
                        nc.tensor.matmul(
                            g_ps[:, sl], hT_sb[:, ic, :], wg_sb[:, ic, sl],
                            start=(ic == 0), stop=False)

                moT_ps = tps.tile([128, NIC, 128], F32, tag="tp_ps")
                for ic in range(NIC):
                    nc.tensor.transpose(
                        moT_ps[:, ic, :], mo_sb[:, ic * 128:(ic + 1) * 128],
                        eye_sb[:])
                moT_sb = csb.tile([128, NIC, 128], BF16, tag="moT")
                nc.scalar.copy(moT_sb[:], moT_ps[:])

                for ic in range(NIC):
                    for jh in range(NJH):
                        sl = slice(jh * 512, (jh + 1) * 512)
                        nc.tensor.matmul(
                            g_ps[:, sl], moT_sb[:, ic, :], wg_sb[:, NIC + ic, sl],
                            start=False, stop=(ic == NIC - 1))

                gb_sb = csb.tile([128, HID], F32, tag="gb")
                nc.vector.tensor_add(gb_sb[:], g_ps[:], bgb_sb[:])
                # sigmoid(x) = 0.5*tanh(x/2) + 0.5 (tanh shares ACT set w/ exp)
                nc.scalar.activation(gb_sb[:], gb_sb[:], AF.Tanh, scale=0.5)

                # aug = h + g*mo = (h + 0.5*mo) + (0.5*mo)*tanh
                u_sb = csb.tile([128, HID], F32, tag="u")
                nc.vector.scalar_tensor_tensor(
                    u_sb[:], mo_sb[:], 0.5, h_sb[:], op0=OP.mult, op1=OP.add)
                v_sb = csb.tile([128, HID], F32, tag="v")
                nc.vector.scalar_tensor_tensor(
                    v_sb[:], gb_sb[:], 0.5, mo_sb[:], op0=OP.mult, op1=OP.mult)
                nc.vector.scalar_tensor_tensor(
                    u_sb[:], u_sb[:], 0.0, v_sb[:], op0=OP.add, op1=OP.add,
                    accum_out=sum_all[:, t:t + 1])
                # square's tensor output is scrap; we only keep the accumulator
                nc.scalar.activation(
                    v_sb[:], u_sb[:], AF.Square, accum_out=ss_all[:, t:t + 1])

                # ---- LayerNorm finalize, per tile, VectorE only ----
                mean = stp.tile([128, 1], F32, tag="mean")
                nc.vector.tensor_scalar_mul(mean[:], sum_all[:, t:t + 1], 1.0 / HID)
                m2 = stp.tile([128, 1], F32, tag="m2")
                nc.vector.tensor_mul(m2[:], mean[:], mean[:])
                nc.vector.tensor_scalar_add(m2[:], m2[:], -LN_EPS)
                vpe = stp.tile([128, 1], F32, tag="vpe")
                nc.vector.scalar_tensor_tensor(
                    vpe[:], ss_all[:, t:t + 1], 1.0 / HID, m2[:],
                    op0=OP.mult, op1=OP.subtract)
                # rstd = 1/sqrt(vpe): quake init + 3 Newton iterations
                y = stp.tile([128, 1], F32, tag="y")
                yi = y[:].bitcast(I32)
                nc.vector.tensor_scalar(
                    yi, vpe[:].bitcast(I32), 1, None,
                    op0=OP.logical_shift_right)
                nc.vector.tensor_scalar(
                    yi, yi, -RSQRT_MAGIC, -1,
                    op0=OP.add, op1=OP.mult)
                yy = stp.tile([128, 1], F32, tag="yy")
                hw = stp.tile([128, 1], F32, tag="hw")
                for _ in range(3):
                    nc.vector.tensor_mul(yy[:], y[:], y[:])
                    nc.vector.tensor_mul(yy[:], yy[:], vpe[:])
                    nc.vector.tensor_scalar(
                        hw[:], yy[:], -0.5, 1.5, op0=OP.mult, op1=OP.add)
                    nc.vector.tensor_mul(y[:], y[:], hw[:])

                # yout = (aug - mean)*rstd*lng + lnb, split DVE / GpSimd
                nc.vector.scalar_tensor_tensor(
                    u_sb[:], u_sb[:], mean[:], lng_sb[:],
                    op0=OP.subtract, op1=OP.mult)
                yo_sb = csb.tile([128, HID], F32, tag="yo")
                nc.vector.scalar_tensor_tensor(
                    yo_sb[:], u_sb[:], y[:], lnb_sb[:],
                    op0=OP.mult, op1=OP.add)
                nc.sync.dma_start(out_d.ap()[t], yo_sb[:])

        pAO_cm.__exit__(None, None, None)   # release attnout
        pWO_cm.__exit__(None, None, None)   # release Wo

    nc.compile()
    return nc


def _prep_core(hs, mk, mv, nt):
    """Host-side lossless layout prep for one core's shard."""
    hT = np.ascontiguousarray(
        hs.reshape(nt, 128, NIC, 128).transpose(0, 3, 2, 1))      # [t,p,ic,b]
    h = np.ascontiguousarray(hs.reshape(nt, 128, HID))
    mkT = np.ascontiguousarray(
        mk.reshape(nt, 128, TOPK, NIC, 128).transpose(0, 2, 4, 3, 1))
    mvT = np.ascontiguousarray(
        mv.reshape(nt, 128, TOPK, NIC, 128).transpose(0, 2, 4, 3, 1))
    return hT, h, mkT, mvT


def kernel(**inputs):
    hs = np.asarray(inputs["hidden_state"], dtype=np.float32)
    mk = np.asarray(inputs["memory_keys"], dtype=np.float32)
    mv = np.asarray(inputs["memory_values"], dtype=np.float32)

    import ml_dtypes
    bf = ml_dtypes.bfloat16
    wq = np.ascontiguousarray(
        np.asarray(inputs["Wq"], np.float32).reshape(NIC, 128, HID).transpose(1, 0, 2)).astype(bf)
    wk = np.ascontiguousarray(
        np.asarray(inputs["Wk"], np.float32).reshape(NIC, 128, HID).transpose(1, 0, 2)).astype(bf)
    wv = np.ascontiguousarray(
        np.asarray(inputs["Wv"], np.float32).reshape(NIC, 128, HID).transpose(1, 0, 2)).astype(bf)
    wo = np.ascontiguousarray(
        np.asarray(inputs["Wo"], np.float32).reshape(NIC, 128, HID).transpose(1, 0, 2))
    wg = np.ascontiguousarray(
        np.asarray(inputs["Wg"], np.float32).reshape(2 * NIC, 128, HID).transpose(1, 0, 2)).astype(bf)
    bgb = np.ascontiguousarray(
        np.broadcast_to(np.asarray(inputs["bg"], np.float32), (128, HID)))
    lng = np.ascontiguousarray(
        np.broadcast_to(np.asarray(inputs["ln_g"], np.float32), (128, HID)))
    lnb = np.ascontiguousarray(
        np.broadcast_to(np.asarray(inputs["ln_b"], np.float32), (128, HID)))
    eye = np.eye(128, dtype=np.float32)

    if "nc" not in _CACHE:
        _CACHE["nc"] = _build(NT)
    nc = _CACHE["nc"]

    in_maps = []
    for c in range(N_CORES):
        sl = slice(c * BC, (c + 1) * BC)
        hT, h, mkT, mvT = _prep_core(hs[sl], mk[sl], mv[sl], NT)
        in_maps.append({
            "hTb": hT.astype(bf), "h": h,
            "mkT": mkT.astype(bf), "mvT": mvT.astype(bf),
            "Wq": wq, "Wk": wk, "Wv": wv, "Wo": wo, "Wg": wg,
            "bgB": bgb, "eye": eye, "lngB": lng, "lnbB": lnb,
        })

    res = run_bass_kernel_spmd(nc, in_maps, core_ids=list(range(N_CORES)),
                               trace=TRACE)
    kernel.last_result = res
    out = np.concatenate(
        [r["out"].reshape(BC, HID) for r in res.results], axis=0)
    return out


kernel.last_result = None



# revision 9
# speedup vs baseline: 1.0244x; 1.0244x over previous
"""Trainium2 Bass kernel for nn_EngramModule: single-query top-k memory attention
with gated residual + LayerNorm, data-parallel across 8 NeuronCores.

Contract: kernel(**inputs) takes the FULL unsharded inputs and returns the FULL
(8192, 1024) float32 output.

Per-core pipeline (1024 batch rows, 8 row-tiles of 128):
  A+B (merged, software-pipelined per tile): Q = h @ Wq in bf16, then per
      k-slot: Kp projection in fp8 e4m3 DoubleRow mode (256-row contraction
      per instruction, ~2x bf16 throughput; Wk pre-scaled by 32 on host to
      dodge e4m3 subnormals, folded back in the exp scale); Vp projection in
      bf16 (error budget: V feeds the output directly, the K path only
      perturbs softmax weights).  Kp is evicted PSUM->SBUF as bf16 by
      ScalarE so the score multiply+reduce run the DVE 2x packed mode;
      attn-weighted V accumulates in f32 with GpSimd adds (its only job).
  C:  memory_out = attnout @ Wo in bf16.  The gate's mo-branch is folded on
      host: mo @ Wg2 = ao @ (Wo @ Wg2) = ao @ Wog, so no mo transpose or
      second eviction chain exists; gate = hT8 @ Wg1 + aoT8 @ Wog, both in
      fp8 DoubleRow (weights *32, folded into the tanh scale).
      sigmoid(x) = 0.5*tanh(x/2)+0.5 keeps ScalarE in one ACT table set;
      aug = h + g*mo; LayerNorm finalizes per tile with a VectorE-only
      Newton rsqrt (bit-trick init).

Bulk weight streams ride the ScalarE HWDGE queue; latency-critical per-tile
loads ride the SyncE queue so they are never stuck behind megabyte weight
transfers. Activations are pre-laid-out on host (pure transpose/reshape,
lossless) so the contraction dim sits on SBUF partitions and no on-chip
transposes of the big tensors are needed.
"""

import os
import sys

import numpy as np

for _p in ("/opt/trn_rl_repo", "/root/.axon_site/_ro/trn_rl_repo"):
    if os.path.isdir(_p) and _p not in sys.path:
        sys.path.insert(0, _p)

from contextlib import ExitStack

import concourse.bacc as bacc
import concourse.mybir as mybir
import concourse.tile as tile
from concourse.bass_utils import run_bass_kernel_spmd

F32 = mybir.dt.float32
F32R = mybir.dt.float32r
BF16 = mybir.dt.bfloat16
F8 = mybir.dt.float8e4
I32 = mybir.dt.int32
AX = mybir.AxisListType
OP = mybir.AluOpType
AF = mybir.ActivationFunctionType
DR = mybir.MatmulPerfMode.DoubleRow

N_CORES = 8
B = 8192
HID = 1024
NH = 16
DH = 64
TOPK = 8
LN_EPS = 1e-5

BC = B // N_CORES          # rows per core = 1024
NT = BC // 128             # row-tiles per core = 8
NIC = HID // 128           # 128-row contraction chunks = 8
NICP = NIC // 2            # fp8 DoubleRow 256-row chunk pairs = 4
NJH = HID // 512           # 512-wide output halves = 2
SCALE = DH ** -0.5
WS = 32.0                  # host pre-scale on fp8 weights (power of 2)
RSQRT_MAGIC = 0x5F3759DF

# Set by test.py to collect a profile; grading path leaves this off.
TRACE = False

_CACHE = {}


def _build(nt=NT):
    nc = bacc.Bacc("TRN2", target_bir_lowering=False, debug=False,
                   num_devices=N_CORES)

    # ---- DRAM parameters (per-core shard, host-prepped layouts) ----
    h_d = nc.declare_dram_parameter("h", [nt, 128, HID], F32, isOutput=False)
    mkT_d = nc.declare_dram_parameter("mkT", [nt, TOPK, 128, NIC, 128], F8, isOutput=False)
    mvT_d = nc.declare_dram_parameter("mvT", [nt, TOPK, 128, NIC, 128], BF16, isOutput=False)
    hTb_d = nc.declare_dram_parameter("hTb", [nt, 128, NIC, 128], BF16, isOutput=False)
    hT8_d = nc.declare_dram_parameter("hT8", [nt, 128, NIC, 128], F8, isOutput=False)
    wq_d = nc.declare_dram_parameter("Wq", [128, NIC, HID], BF16, isOutput=False)
    wk_d = nc.declare_dram_parameter("Wk", [128, NIC, HID], F8, isOutput=False)
    wv_d = nc.declare_dram_parameter("Wv", [128, NIC, HID], BF16, isOutput=False)
    wo_d = nc.declare_dram_parameter("Wo", [128, NIC, HID], BF16, isOutput=False)
    wg1_d = nc.declare_dram_parameter("Wg1", [128, NIC, HID], F8, isOutput=False)
    wog_d = nc.declare_dram_parameter("Wog", [128, NIC, HID], BF16, isOutput=False)
    bgb_d = nc.declare_dram_parameter("bgB", [128, HID], F32, isOutput=False)
    eye_d = nc.declare_dram_parameter("eye", [128, 128], F32, isOutput=False)
    lng_d = nc.declare_dram_parameter("lngB", [128, HID], F32, isOutput=False)
    lnb_d = nc.declare_dram_parameter("lnbB", [128, HID], F32, isOutput=False)
    out_d = nc.declare_dram_parameter("out", [nt, 128, HID], F32, isOutput=True)

    def load_w(tile_sb, dram, nchunk):
        # bulk weights on the ScalarE HWDGE queue, chunked so the first
        # dependent matmul only waits for its own chunk
        for ic in range(nchunk):
            nc.scalar.dma_start(tile_sb[:, ic, :], dram.ap()[:, ic, :])

    with ExitStack() as octx:
        tc = octx.enter_context(tile.TileContext(nc))

        pers = octx.enter_context(tc.tile_pool(name="pers", bufs=1))
        sum_all = pers.tile([128, nt], F32, tag="sum_all")
        ss_all = pers.tile([128, nt], F32, tag="ss_all")

        # long-lived weights / cross-phase activations
        pWO_cm = tc.tile_pool(name="pWO", bufs=1); pWO = pWO_cm.__enter__()
        wo_sb = pWO.tile([128, NIC, HID], BF16, tag="wo")
        wg1_sb = pWO.tile([128, NIC, HID], F8, tag="wg1")
        wog_sb = pWO.tile([128, NIC, HID], BF16, tag="wog")
        eye_sb = pWO.tile([128, 128], F32, tag="eye")

        pAO_cm = tc.tile_pool(name="pAO", bufs=1); pAO = pAO_cm.__enter__()
        ao_all = pAO.tile([128, nt, HID], F32, tag="ao_all")

        pAB_cm = tc.tile_pool(name="pAB", bufs=1); pAB = pAB_cm.__enter__()
        q_all = pAB.tile([128, nt, HID], BF16, tag="q_all")

        pWKV_cm = tc.tile_pool(name="wkv", bufs=1); wkv = pWKV_cm.__enter__()
        wq_sb = wkv.tile([128, NIC, HID], BF16, tag="wq")
        wk_sb = wkv.tile([128, NIC, HID], F8, tag="wk")
        wv_sb = wkv.tile([128, NIC, HID], BF16, tag="wv")

        # ============ merged phase A+B: Q projection + attention ============
        with ExitStack() as bctx:
            hp = bctx.enter_context(tc.tile_pool(name="hT_a", bufs=1))
            mp = bctx.enter_context(tc.tile_pool(name="mkv", bufs=4))
            kvps = bctx.enter_context(tc.tile_pool(name="kv_ps", bufs=2, space="PSUM"))
            kbp = bctx.enter_context(tc.tile_pool(name="kb", bufs=2))
            sp = bctx.enter_context(tc.tile_pool(name="scr", bufs=2))
            accp = bctx.enter_context(tc.tile_pool(name="acc", bufs=2))
            ep = bctx.enter_context(tc.tile_pool(name="e", bufs=2))

            # scalar HWDGE queue, in consumption order
            load_w(wq_sb, wq_d, NIC)
            load_w(wk_sb, wk_d, NIC)
            load_w(wv_sb, wv_d, NIC)
            load_w(wo_sb, wo_d, NIC)
            load_w(wg1_sb, wg1_d, NIC)
            load_w(wog_sb, wog_d, NIC)
            nc.scalar.dma_start(eye_sb[:], eye_d.ap())

            # resident hT (bf16, for Q); sync queue ahead of the mk/mv stream
            hT_all = hp.tile([128, nt, NIC, 128], BF16, tag="hT")
            for t in range(nt):
                nc.sync.dma_start(hT_all[:, t], hTb_d.ap()[t])
            preload = {}
            for (pt, pk) in ((0, 0),):
                a = mp.tile([128, NIC, 128], F8, tag="mkT")
                nc.sync.dma_start(a[:], mkT_d.ap()[pt, pk])
                b_ = mp.tile([128, NIC, 128], BF16, tag="mvT")
                nc.sync.dma_start(b_[:], mvT_d.ap()[pt, pk])
                preload[(pt, pk)] = (a, b_)

            for t in range(nt):
                # ---- Q projection for this tile (PSUM slot shared w/ kp) ----
                q_ps = kvps.tile([128, HID], F32, tag="kp")
                for ic in range(NIC):
                    for jh in range(NJH):
                        nc.tensor.matmul(
                            q_ps[:, jh * 512:(jh + 1) * 512],
                            hT_all[:, t, ic, :],
                            wq_sb[:, ic, jh * 512:(jh + 1) * 512],
                            start=(ic == 0), stop=(ic == NIC - 1),
                        )
                nc.scalar.copy(q_all[:, t, :], q_ps[:])

                acc = accp.tile([128, HID], F32, tag="acc")
                e_all = ep.tile([128, TOPK, NH], BF16, tag="e_all")
                q_t = q_all[:, t, :]
                for k in range(TOPK):
                    if (t, k) in preload:
                        mkT, mvT = preload[(t, k)]
                    else:
                        mkT = mp.tile([128, NIC, 128], F8, tag="mkT")
                        nc.sync.dma_start(mkT[:], mkT_d.ap()[t, k])
                        mvT = mp.tile([128, NIC, 128], BF16, tag="mvT")
                        nc.sync.dma_start(mvT[:], mvT_d.ap()[t, k])

                    kp_ps = kvps.tile([128, HID], F32, tag="kp")
                    vp_ps = kvps.tile([128, HID], F32, tag="vp")
                    # K projection: fp8 DoubleRow, 256-row contraction pairs
                    for ip in range(NICP):
                        for jh in range(NJH):
                            nc.tensor.matmul(
                                kp_ps[:, jh * 512:(jh + 1) * 512],
                                mkT[:, 2 * ip:2 * ip + 2, :],
                                wk_sb[:, 2 * ip:2 * ip + 2, jh * 512:(jh + 1) * 512],
                                start=(ip == 0), stop=(ip == NICP - 1),
                                perf_mode=DR,
                            )
                    for ic in range(NIC):
                        for jh in range(NJH):
                            nc.tensor.matmul(
                                vp_ps[:, jh * 512:(jh + 1) * 512],
                                mvT[:, ic, :],
                                wv_sb[:, ic, jh * 512:(jh + 1) * 512],
                                start=(ic == 0), stop=(ic == NIC - 1),
                            )

                    # Kp -> bf16 SBUF so the DVE score chain runs 2x packed
                    kp_bf = kbp.tile([128, HID], BF16, tag="kb")
                    nc.scalar.copy(kp_bf[:], kp_ps[:])

                    # scores for all 16 heads of this k-slot
                    p_scr = sp.tile([128, HID], BF16, tag="p")
                    nc.vector.tensor_mul(p_scr[:], q_t, kp_bf[:])
                    s_k = ep.tile([128, NH], BF16, tag="s_k")
                    # bf16 store keeps the reduce in the DVE 2x packed mode;
                    # the accumulate itself is fp32 internally
                    with nc.allow_low_precision(reason="bf16 score store"):
                        nc.vector.reduce_sum(
                            s_k[:], p_scr[:].rearrange("p (h d) -> p h d", h=NH),
                            axis=AX.X)
                    # e = exp(scores * DH**-0.5 / WS); logits ~N(0,1) so no
                    # max-sub; /WS folds out the host Wk pre-scale
                    nc.scalar.activation(e_all[:, k, :], s_k[:], AF.Exp,
                                         scale=SCALE / WS)

                    # weighted V accumulate: acc += e[:,k,h] (bcast over d) * Vp
                    e_bc = e_all[:, k, :].unsqueeze(2).broadcast_to([128, NH, DH])
                    dst = acc if k == 0 else sp.tile([128, HID], F32, tag="pv")
                    nc.vector.tensor_tensor(
                        dst[:].rearrange("p (h d) -> p h d", h=NH),
                        vp_ps[:].rearrange("p (h d) -> p h d", h=NH),
                        e_bc, op=OP.mult)
                    if k > 0:
                        nc.gpsimd.tensor_add(acc[:], acc[:], dst[:])

                # normalize: attnout = acc * (1/sum_k e)
                den = ep.tile([128, NH], F32, tag="den")
                nc.vector.reduce_sum(
                    den[:], e_all[:].rearrange("p k h -> p h k"), axis=AX.X)
                rden = ep.tile([128, NH], F32, tag="rden")
                nc.vector.reciprocal(rden[:], den[:])
                rden_bc = rden[:].unsqueeze(2).broadcast_to([128, NH, DH])
                nc.vector.tensor_tensor(
                    ao_all[:, t, :].rearrange("p (h d) -> p h d", h=NH),
                    acc[:].rearrange("p (h d) -> p h d", h=NH),
                    rden_bc, op=OP.mult)

        pWKV_cm.__exit__(None, None, None)  # release Wq/Wk/Wv
        pAB_cm.__exit__(None, None, None)   # release q_all

        # ===== phase C: memory_out, gate, residual, LayerNorm, output =====
        with ExitStack() as cctx:
            eyep = cctx.enter_context(tc.tile_pool(name="cconst", bufs=1))
            cstr = cctx.enter_context(tc.tile_pool(name="c_str", bufs=2))
            csb = cctx.enter_context(tc.tile_pool(name="c_sb", bufs=2))
            stp = cctx.enter_context(tc.tile_pool(name="stats", bufs=2))
            tps = cctx.enter_context(tc.tile_pool(name="tp_ps", bufs=1, space="PSUM"))
            mps = cctx.enter_context(tc.tile_pool(name="mo_ps", bufs=1, space="PSUM"))
            gps = cctx.enter_context(tc.tile_pool(name="g_ps", bufs=2, space="PSUM"))

            # epilogue constants: scalar queue is idle by now
            bgb_sb = eyep.tile([128, HID], F32, tag="bgb")
            nc.scalar.dma_start(bgb_sb[:], bgb_d.ap())
            lng_sb = eyep.tile([128, HID], F32, tag="lng")
            nc.scalar.dma_start(lng_sb[:], lng_d.ap())
            lnb_sb = eyep.tile([128, HID], F32, tag="lnb")
            nc.scalar.dma_start(lnb_sb[:], lnb_d.ap())

            for t in range(nt):
                hT_sb = cstr.tile([128, NIC, 128], F8, tag="hT_c")
                nc.scalar.dma_start(hT_sb[:], hT8_d.ap()[t])
                h_sb = cstr.tile([128, HID], F32, tag="h_c")
                nc.scalar.dma_start(h_sb[:], h_d.ap()[t])

                # attn-out transposes, then both evictions (bf16 for Wo,
                # fp8 for the gate's Wog branch)
                at_ps = tps.tile([128, NIC, 128], F32, tag="tp_ps")
                for ic in range(NIC):
                    nc.tensor.transpose(
                        at_ps[:, ic, :], ao_all[:, t, ic * 128:(ic + 1) * 128],
                        eye_sb[:])
                atT_sb = csb.tile([128, NIC, 128], BF16, tag="atT")
                nc.scalar.copy(atT_sb[:], at_ps[:])

                # gate h-branch: fp8 DoubleRow; fills TensorE while the
                # evictions drain on ScalarE
                g_ps = gps.tile([128, HID], F32, tag="g_ps")
                for ip in range(NICP):
                    for jh in range(NJH):
                        sl = slice(jh * 512, (jh + 1) * 512)
                        nc.tensor.matmul(
                            g_ps[:, sl], hT_sb[:, 2 * ip:2 * ip + 2, :],
                            wg1_sb[:, 2 * ip:2 * ip + 2, sl],
                            start=(ip == 0), stop=False, perf_mode=DR)

                mo_ps = mps.tile([128, HID], F32, tag="mo_ps")
                for ic in range(NIC):
                    for jh in range(NJH):
                        nc.tensor.matmul(
                            mo_ps[:, jh * 512:(jh + 1) * 512],
                            atT_sb[:, ic, :],
                            wo_sb[:, ic, jh * 512:(jh + 1) * 512],
                            start=(ic == 0), stop=(ic == NIC - 1),
                        )

                # gate ao-branch: ao @ (Wo @ Wg2) folded on host (bf16: fp8
                # noise here spikes the max-err tail, and the fold already
                # removed the transpose chain)
                for ic in range(NIC):
                    for jh in range(NJH):
                        sl = slice(jh * 512, (jh + 1) * 512)
                        nc.tensor.matmul(
                            g_ps[:, sl], atT_sb[:, ic, :],
                            wog_sb[:, ic, sl],
                            start=False, stop=(ic == NIC - 1))

                mo_sb = csb.tile([128, HID], F32, tag="mo")
                nc.scalar.copy(mo_sb[:], mo_ps[:])

                gb_sb = csb.tile([128, HID], F32, tag="gb")
                nc.vector.tensor_add(gb_sb[:], g_ps[:], bgb_sb[:])
                # sigmoid(x) = 0.5*tanh(x/2) + 0.5 (tanh shares ACT set w/ exp)
                # scale folds out the host Wg pre-scale
                nc.scalar.activation(gb_sb[:], gb_sb[:], AF.Tanh,
                                     scale=0.5 / WS)

                # aug = h + g*mo = (h + 0.5*mo) + (0.5*mo)*tanh
                u_sb = csb.tile([128, HID], F32, tag="u")
                nc.vector.scalar_tensor_tensor(
                    u_sb[:], mo_sb[:], 0.5, h_sb[:], op0=OP.mult, op1=OP.add)
                v_sb = csb.tile([128, HID], F32, tag="v")
                nc.vector.scalar_tensor_tensor(
                    v_sb[:], gb_sb[:], 0.5, mo_sb[:], op0=OP.mult, op1=OP.mult)
                nc.vector.scalar_tensor_tensor(
                    u_sb[:], u_sb[:], 0.0, v_sb[:], op0=OP.add, op1=OP.add,
                    accum_out=sum_all[:, t:t + 1])
                # square's tensor output is scrap; we only keep the accumulator
                nc.scalar.activation(
                    v_sb[:], u_sb[:], AF.Square, accum_out=ss_all[:, t:t + 1])

                # ---- LayerNorm finalize, per tile, VectorE only ----
                mean = stp.tile([128, 1], F32, tag="mean")
                nc.vector.tensor_scalar_mul(mean[:], sum_all[:, t:t + 1], 1.0 / HID)
                m2 = stp.tile([128, 1], F32, tag="m2")
                nc.vector.tensor_mul(m2[:], mean[:], mean[:])
                nc.vector.tensor_scalar_add(m2[:], m2[:], -LN_EPS)
                vpe = stp.tile([128, 1], F32, tag="vpe")
                nc.vector.scalar_tensor_tensor(
                    vpe[:], ss_all[:, t:t + 1], 1.0 / HID, m2[:],
                    op0=OP.mult, op1=OP.subtract)
                # rstd = 1/sqrt(vpe): quake init + 3 Newton iterations
                y = stp.tile([128, 1], F32, tag="y")
                yi = y[:].bitcast(I32)
                nc.vector.tensor_scalar(
                    yi, vpe[:].bitcast(I32), 1, None,
                    op0=OP.logical_shift_right)
                nc.vector.tensor_scalar(
                    yi, yi, -RSQRT_MAGIC, -1,
                    op0=OP.add, op1=OP.mult)
                yy = stp.tile([128, 1], F32, tag="yy")
                hw = stp.tile([128, 1], F32, tag="hw")
                for _ in range(3):
                    nc.vector.tensor_mul(yy[:], y[:], y[:])
                    nc.vector.tensor_mul(yy[:], yy[:], vpe[:])
                    nc.vector.tensor_scalar(
                        hw[:], yy[:], -0.5, 1.5, op0=OP.mult, op1=OP.add)
                    nc.vector.tensor_mul(y[:], y[:], hw[:])

                # yout = (aug - mean)*rstd*lng + lnb
                nc.vector.scalar_tensor_tensor(
                    u_sb[:], u_sb[:], mean[:], lng_sb[:],
                    op0=OP.subtract, op1=OP.mult)
                yo_sb = csb.tile([128, HID], F32, tag="yo")
                nc.vector.scalar_tensor_tensor(
                    yo_sb[:], u_sb[:], y[:], lnb_sb[:],
                    op0=OP.mult, op1=OP.add)
                nc.sync.dma_start(out_d.ap()[t], yo_sb[:])

        pAO_cm.__exit__(None, None, None)   # release attnout
        pWO_cm.__exit__(None, None, None)   # release Wo/Wg/Wog/eye

    nc.compile()
    return nc


def _prep_core(hs, mk, mv, nt):
    """Host-side lossless layout prep for one core's shard."""
    hT = np.ascontiguousarray(
        hs.reshape(nt, 128, NIC, 128).transpose(0, 3, 2, 1))      # [t,p,ic,b]
    h = np.ascontiguousarray(hs.reshape(nt, 128, HID))
    mkT = np.ascontiguousarray(
        mk.reshape(nt, 128, TOPK, NIC, 128).transpose(0, 2, 4, 3, 1))
    mvT = np.ascontiguousarray(
        mv.reshape(nt, 128, TOPK, NIC, 128).transpose(0, 2, 4, 3, 1))
    return hT, h, mkT, mvT


def kernel(**inputs):
    hs = np.asarray(inputs["hidden_state"], dtype=np.float32)
    mk = np.asarray(inputs["memory_keys"], dtype=np.float32)
    mv = np.asarray(inputs["memory_values"], dtype=np.float32)

    import ml_dtypes
    bf = ml_dtypes.bfloat16
    f8 = ml_dtypes.float8_e4m3

    def wlay(w):
        return np.ascontiguousarray(
            np.asarray(w, np.float32).reshape(NIC, 128, HID).transpose(1, 0, 2))

    wg_full = np.asarray(inputs["Wg"], np.float32)
    wo_f32 = np.asarray(inputs["Wo"], np.float32)
    wog_f32 = wo_f32 @ wg_full[HID:]          # fold Wo into the gate branch

    wq = wlay(inputs["Wq"]).astype(bf)
    wk = (wlay(inputs["Wk"]) * WS).astype(f8)
    wv = wlay(inputs["Wv"]).astype(bf)
    wo = wlay(wo_f32).astype(bf)
    wg1 = (wlay(wg_full[:HID]) * WS).astype(f8)
    wog = (wlay(wog_f32) * WS).astype(bf)
    bgb = np.ascontiguousarray(
        np.broadcast_to(np.asarray(inputs["bg"], np.float32) * WS, (128, HID)))
    lng = np.ascontiguousarray(
        np.broadcast_to(np.asarray(inputs["ln_g"], np.float32), (128, HID)))
    lnb = np.ascontiguousarray(
        np.broadcast_to(np.asarray(inputs["ln_b"], np.float32), (128, HID)))
    eye = np.eye(128, dtype=np.float32)

    if "nc" not in _CACHE:
        _CACHE["nc"] = _build(NT)
    nc = _CACHE["nc"]

    in_maps = []
    for c in range(N_CORES):
        sl = slice(c * BC, (c + 1) * BC)
        hT, h, mkT, mvT = _prep_core(hs[sl], mk[sl], mv[sl], NT)
        in_maps.append({
            "hTb": hT.astype(bf), "hT8": hT.astype(f8), "h": h,
            "mkT": mkT.astype(f8), "mvT": mvT.astype(bf),
            "Wq": wq, "Wk": wk, "Wv": wv, "Wo": wo, "Wg1": wg1, "Wog": wog,
            "bgB": bgb, "eye": eye, "lngB": lng, "lnbB": lnb,
        })

    res = run_bass_kernel_spmd(nc, in_maps, core_ids=list(range(N_CORES)),
                               trace=TRACE)
    kernel.last_result = res
    out = np.concatenate(
        [r["out"].reshape(BC, HID) for r in res.results], axis=0)
    return out


kernel.last_result = None


# revision 12
# speedup vs baseline: 1.2005x; 1.1719x over previous
"""Trainium2 Bass kernel for nn_EngramModule: single-query top-k memory attention
with gated residual + LayerNorm, data-parallel across 8 NeuronCores.

Contract: kernel(**inputs) takes the FULL unsharded inputs and returns the FULL
(8192, 1024) float32 output.

Per-core pipeline (1024 batch rows, 8 row-tiles of 128):
  A+B (merged, software-pipelined per tile): Q = h @ Wq in bf16, then per
      k-slot: Kp projection in fp8 e4m3 DoubleRow mode (256-row contraction
      per instruction, ~2x bf16 throughput; Wk pre-scaled by 32 on host to
      dodge e4m3 subnormals, folded back in the exp scale); Vp projection in
      bf16 (error budget: V feeds the output directly, the K path only
      perturbs softmax weights).  Kp is evicted PSUM->SBUF as bf16 by
      ScalarE so the score multiply+reduce run the DVE 2x packed mode;
      attn-weighted V accumulates in f32 with GpSimd adds (its only job).
  C:  memory_out = attnout @ Wo in bf16.  The gate's mo-branch is folded on
      host: mo @ Wg2 = ao @ (Wo @ Wg2) = ao @ Wog, so no mo transpose or
      second eviction chain exists; gate = hT8 @ Wg1 + aoT8 @ Wog, both in
      fp8 DoubleRow (weights *32, folded into the tanh scale).
      sigmoid(x) = 0.5*tanh(x/2)+0.5 keeps ScalarE in one ACT table set;
      aug = h + g*mo; LayerNorm finalizes per tile with a VectorE-only
      Newton rsqrt (bit-trick init).

Bulk weight streams ride the ScalarE HWDGE queue; latency-critical per-tile
loads ride the SyncE queue so they are never stuck behind megabyte weight
transfers. Activations are pre-laid-out on host (pure transpose/reshape,
lossless) so the contraction dim sits on SBUF partitions and no on-chip
transposes of the big tensors are needed.
"""

import os
import sys

import numpy as np

for _p in ("/opt/trn_rl_repo", "/root/.axon_site/_ro/trn_rl_repo"):
    if os.path.isdir(_p) and _p not in sys.path:
        sys.path.insert(0, _p)

from contextlib import ExitStack

import concourse.bacc as bacc
import concourse.mybir as mybir
import concourse.tile as tile
from concourse.bass_utils import run_bass_kernel_spmd

F32 = mybir.dt.float32
F32R = mybir.dt.float32r
BF16 = mybir.dt.bfloat16
F8 = mybir.dt.float8e4
I32 = mybir.dt.int32
AX = mybir.AxisListType
OP = mybir.AluOpType
AF = mybir.ActivationFunctionType
DR = mybir.MatmulPerfMode.DoubleRow

N_CORES = 8
B = 8192
HID = 1024
NH = 16
DH = 64
TOPK = 8
LN_EPS = 1e-5

BC = B // N_CORES          # rows per core = 1024
NT = BC // 128             # row-tiles per core = 8
NIC = HID // 128           # 128-row contraction chunks = 8
NICP = NIC // 2            # fp8 DoubleRow 256-row chunk pairs = 4
NJH = HID // 512           # 512-wide output halves = 2
SCALE = DH ** -0.5
WS = 32.0                  # host pre-scale on fp8 weights (power of 2)
RSQRT_MAGIC = 0x5F3759DF

# Set by test.py to collect a profile; grading path leaves this off.
TRACE = False

_CACHE = {}


def _build(nt=NT):
    nc = bacc.Bacc("TRN2", target_bir_lowering=False, debug=False,
                   num_devices=N_CORES)

    # ---- DRAM parameters (per-core shard, host-prepped layouts) ----
    h_d = nc.declare_dram_parameter("h", [nt, 128, HID], F32, isOutput=False)
    mkT_d = nc.declare_dram_parameter("mkT", [nt, TOPK, 128, NIC, 128], F8, isOutput=False)
    mvT_d = nc.declare_dram_parameter("mvT", [nt, TOPK, 128, NIC, 128], BF16, isOutput=False)
    hTb_d = nc.declare_dram_parameter("hTb", [nt, 128, NIC, 128], BF16, isOutput=False)
    hT8_d = nc.declare_dram_parameter("hT8", [nt, 128, NIC, 128], F8, isOutput=False)
    wq_d = nc.declare_dram_parameter("Wq", [128, NIC, HID], BF16, isOutput=False)
    wk_d = nc.declare_dram_parameter("Wk", [128, NIC, HID], F8, isOutput=False)
    wv_d = nc.declare_dram_parameter("Wv", [128, NIC, HID], BF16, isOutput=False)
    wo_d = nc.declare_dram_parameter("Wo", [128, NIC, HID], BF16, isOutput=False)
    wg1_d = nc.declare_dram_parameter("Wg1", [128, NIC, HID], F8, isOutput=False)
    wog_d = nc.declare_dram_parameter("Wog", [128, NIC, HID], BF16, isOutput=False)
    bgb_d = nc.declare_dram_parameter("bgB", [128, HID], F32, isOutput=False)
    eye_d = nc.declare_dram_parameter("eye", [128, 128], F32, isOutput=False)
    lng_d = nc.declare_dram_parameter("lngB", [128, HID], F32, isOutput=False)
    lnb_d = nc.declare_dram_parameter("lnbB", [128, HID], F32, isOutput=False)
    out_d = nc.declare_dram_parameter("out", [nt, 128, HID], F32, isOutput=True)

    def load_w(tile_sb, dram, nchunk):
        # bulk weights on the ScalarE HWDGE queue, chunked so the first
        # dependent matmul only waits for its own chunk
        for ic in range(nchunk):
            nc.scalar.dma_start(tile_sb[:, ic, :], dram.ap()[:, ic, :])

    with ExitStack() as octx:
        tc = octx.enter_context(tile.TileContext(nc))

        pers = octx.enter_context(tc.tile_pool(name="pers", bufs=1))
        sum_all = pers.tile([128, nt], F32, tag="sum_all")
        ss_all = pers.tile([128, nt], F32, tag="ss_all")

        # long-lived weights / cross-phase activations
        pWO_cm = tc.tile_pool(name="pWO", bufs=1); pWO = pWO_cm.__enter__()
        wo_sb = pWO.tile([128, NIC, HID], BF16, tag="wo")
        wg1_sb = pWO.tile([128, NIC, HID], F8, tag="wg1")
        wog_sb = pWO.tile([128, NIC, HID], BF16, tag="wog")
        eye_sb = pWO.tile([128, 128], F32, tag="eye")

        pAO_cm = tc.tile_pool(name="pAO", bufs=1); pAO = pAO_cm.__enter__()
        ao_all = pAO.tile([128, nt, HID], F32, tag="ao_all")

        pAB_cm = tc.tile_pool(name="pAB", bufs=1); pAB = pAB_cm.__enter__()
        q_all = pAB.tile([128, nt, HID], BF16, tag="q_all")

        pWKV_cm = tc.tile_pool(name="wkv", bufs=1); wkv = pWKV_cm.__enter__()
        wq_sb = wkv.tile([128, NIC, HID], BF16, tag="wq")
        wk_sb = wkv.tile([128, NIC, HID], F8, tag="wk")
        wv_sb = wkv.tile([128, NIC, HID], BF16, tag="wv")

        # ============ merged phase A+B: Q projection + attention ============
        with ExitStack() as bctx:
            hp = bctx.enter_context(tc.tile_pool(name="hT_a", bufs=1))
            mp = bctx.enter_context(tc.tile_pool(name="mkv", bufs=4))
            kvps = bctx.enter_context(tc.tile_pool(name="kv_ps", bufs=2, space="PSUM"))
            sp = bctx.enter_context(tc.tile_pool(name="scr", bufs=2))
            accp = bctx.enter_context(tc.tile_pool(name="acc", bufs=2))
            ep = bctx.enter_context(tc.tile_pool(name="e", bufs=2))

            # scalar HWDGE queue, in consumption order
            load_w(wq_sb, wq_d, NIC)
            load_w(wk_sb, wk_d, NIC)
            load_w(wv_sb, wv_d, NIC)
            load_w(wo_sb, wo_d, NIC)
            load_w(wg1_sb, wg1_d, NIC)
            load_w(wog_sb, wog_d, NIC)
            nc.scalar.dma_start(eye_sb[:], eye_d.ap())

            # resident hT (bf16, for Q); sync queue ahead of the mk/mv stream
            hT_all = hp.tile([128, nt, NIC, 128], BF16, tag="hT")
            for t in range(nt):
                nc.sync.dma_start(hT_all[:, t], hTb_d.ap()[t])
            preload = {}
            for (pt, pk) in ((0, 0),):
                a = mp.tile([128, NIC, 128], F8, tag="mkT")
                nc.sync.dma_start(a[:], mkT_d.ap()[pt, pk])
                b_ = mp.tile([128, NIC, 128], BF16, tag="mvT")
                nc.sync.dma_start(b_[:], mvT_d.ap()[pt, pk])
                preload[(pt, pk)] = (a, b_)

            for t in range(nt):
                # ---- Q projection for this tile (PSUM slot shared w/ kp) ----
                q_ps = kvps.tile([128, HID], F32, tag="kp")
                for ic in range(NIC):
                    for jh in range(NJH):
                        nc.tensor.matmul(
                            q_ps[:, jh * 512:(jh + 1) * 512],
                            hT_all[:, t, ic, :],
                            wq_sb[:, ic, jh * 512:(jh + 1) * 512],
                            start=(ic == 0), stop=(ic == NIC - 1),
                        )
                nc.scalar.copy(q_all[:, t, :], q_ps[:])

                acc = accp.tile([128, HID], BF16, tag="acc")
                e_all = ep.tile([128, TOPK, NH], BF16, tag="e_all")
                q_t = q_all[:, t, :]
                for k in range(TOPK):
                    if (t, k) in preload:
                        mkT, mvT = preload[(t, k)]
                    else:
                        mkT = mp.tile([128, NIC, 128], F8, tag="mkT")
                        nc.sync.dma_start(mkT[:], mkT_d.ap()[t, k])
                        mvT = mp.tile([128, NIC, 128], BF16, tag="mvT")
                        nc.sync.dma_start(mvT[:], mvT_d.ap()[t, k])

                    kp_ps = kvps.tile([128, HID], F32, tag="kp")
                    vp_ps = kvps.tile([128, HID], F32, tag="vp")
                    # K projection: fp8 DoubleRow, 256-row contraction pairs
                    for ip in range(NICP):
                        for jh in range(NJH):
                            nc.tensor.matmul(
                                kp_ps[:, jh * 512:(jh + 1) * 512],
                                mkT[:, 2 * ip:2 * ip + 2, :],
                                wk_sb[:, 2 * ip:2 * ip + 2, jh * 512:(jh + 1) * 512],
                                start=(ip == 0), stop=(ip == NICP - 1),
                                perf_mode=DR,
                            )
                    for ic in range(NIC):
                        for jh in range(NJH):
                            nc.tensor.matmul(
                                vp_ps[:, jh * 512:(jh + 1) * 512],
                                mvT[:, ic, :],
                                wv_sb[:, ic, jh * 512:(jh + 1) * 512],
                                start=(ic == 0), stop=(ic == NIC - 1),
                            )

                    # scores for all 16 heads of this k-slot (bf16 products
                    # so the per-head reduce reads packed 16-bit)
                    p_scr = sp.tile([128, HID], BF16, tag="p")
                    nc.vector.tensor_mul(p_scr[:], q_t, kp_ps[:])
                    s_k = ep.tile([128, NH], F32, tag="s_k")
                    nc.vector.reduce_sum(
                        s_k[:], p_scr[:].rearrange("p (h d) -> p h d", h=NH), axis=AX.X)
                    # e = exp(scores * DH**-0.5 / WS); logits ~N(0,1) so no
                    # max-sub; /WS folds out the host Wk pre-scale
                    nc.scalar.activation(e_all[:, k, :], s_k[:], AF.Exp,
                                         scale=SCALE / WS)

                    # weighted V accumulate: acc += e[:,k,h] (bcast over d) * Vp
                    e_bc = e_all[:, k, :].unsqueeze(2).broadcast_to([128, NH, DH])
                    dst = acc if k == 0 else sp.tile([128, HID], BF16, tag="pv")
                    nc.vector.tensor_tensor(
                        dst[:].rearrange("p (h d) -> p h d", h=NH),
                        vp_ps[:].rearrange("p (h d) -> p h d", h=NH),
                        e_bc, op=OP.mult)
                    if k > 0:
                        nc.vector.tensor_add(acc[:], acc[:], dst[:])

                # normalize: attnout = acc * (1/sum_k e)
                den = ep.tile([128, NH], F32, tag="den")
                nc.vector.reduce_sum(
                    den[:], e_all[:].rearrange("p k h -> p h k"), axis=AX.X)
                rden = ep.tile([128, NH], F32, tag="rden")
                nc.vector.reciprocal(rden[:], den[:])
                rden_bc = rden[:].unsqueeze(2).broadcast_to([128, NH, DH])
                nc.vector.tensor_tensor(
                    ao_all[:, t, :].rearrange("p (h d) -> p h d", h=NH),
                    acc[:].rearrange("p (h d) -> p h d", h=NH),
                    rden_bc, op=OP.mult)

        pWKV_cm.__exit__(None, None, None)  # release Wq/Wk/Wv
        pAB_cm.__exit__(None, None, None)   # release q_all

        # ===== phase C: memory_out, gate, residual, LayerNorm, output =====
        with ExitStack() as cctx:
            eyep = cctx.enter_context(tc.tile_pool(name="cconst", bufs=1))
            cstr = cctx.enter_context(tc.tile_pool(name="c_str", bufs=2))
            csb = cctx.enter_context(tc.tile_pool(name="c_sb", bufs=2))
            stp = cctx.enter_context(tc.tile_pool(name="stats", bufs=2))
            tps = cctx.enter_context(tc.tile_pool(name="tp_ps", bufs=1, space="PSUM"))
            mps = cctx.enter_context(tc.tile_pool(name="mo_ps", bufs=1, space="PSUM"))
            gps = cctx.enter_context(tc.tile_pool(name="g_ps", bufs=2, space="PSUM"))

            # epilogue constants: scalar queue is idle by now
            bgb_sb = eyep.tile([128, HID], F32, tag="bgb")
            nc.scalar.dma_start(bgb_sb[:], bgb_d.ap())
            lng_sb = eyep.tile([128, HID], F32, tag="lng")
            nc.scalar.dma_start(lng_sb[:], lng_d.ap())
            lnb_sb = eyep.tile([128, HID], F32, tag="lnb")
            nc.scalar.dma_start(lnb_sb[:], lnb_d.ap())

            for t in range(nt):
                hT_sb = cstr.tile([128, NIC, 128], F8, tag="hT_c")
                nc.scalar.dma_start(hT_sb[:], hT8_d.ap()[t])
                h_sb = cstr.tile([128, HID], F32, tag="h_c")
                nc.scalar.dma_start(h_sb[:], h_d.ap()[t])

                # attn-out transposes, then both evictions (bf16 for Wo,
                # fp8 for the gate's Wog branch)
                at_ps = tps.tile([128, NIC, 128], F32, tag="tp_ps")
                for ic in range(NIC):
                    nc.tensor.transpose(
                        at_ps[:, ic, :], ao_all[:, t, ic * 128:(ic + 1) * 128],
                        eye_sb[:])
                atT_sb = csb.tile([128, NIC, 128], BF16, tag="atT")
                nc.scalar.copy(atT_sb[:], at_ps[:])

                # gate h-branch: fp8 DoubleRow; fills TensorE while the
                # evictions drain on ScalarE
                g_ps = gps.tile([128, HID], F32, tag="g_ps")
                for ip in range(NICP):
                    for jh in range(NJH):
                        sl = slice(jh * 512, (jh + 1) * 512)
                        nc.tensor.matmul(
                            g_ps[:, sl], hT_sb[:, 2 * ip:2 * ip + 2, :],
                            wg1_sb[:, 2 * ip:2 * ip + 2, sl],
                            start=(ip == 0), stop=False, perf_mode=DR)

                mo_ps = mps.tile([128, HID], F32, tag="mo_ps")
                for ic in range(NIC):
                    for jh in range(NJH):
                        nc.tensor.matmul(
                            mo_ps[:, jh * 512:(jh + 1) * 512],
                            atT_sb[:, ic, :],
                            wo_sb[:, ic, jh * 512:(jh + 1) * 512],
                            start=(ic == 0), stop=(ic == NIC - 1),
                        )

                # gate ao-branch: ao @ (Wo @ Wg2) folded on host (bf16: fp8
                # noise here spikes the max-err tail, and the fold already
                # removed the transpose chain)
                for ic in range(NIC):
                    for jh in range(NJH):
                        sl = slice(jh * 512, (jh + 1) * 512)
                        nc.tensor.matmul(
                            g_ps[:, sl], atT_sb[:, ic, :],
                            wog_sb[:, ic, sl],
                            start=False, stop=(ic == NIC - 1))

                mo_sb = csb.tile([128, HID], F32, tag="mo")
                nc.scalar.copy(mo_sb[:], mo_ps[:])

                gb_sb = csb.tile([128, HID], F32, tag="gb")
                nc.vector.tensor_add(gb_sb[:], g_ps[:], bgb_sb[:])
                # sigmoid(x) = 0.5*tanh(x/2) + 0.5 (tanh shares ACT set w/ exp)
                # scale folds out the host Wg pre-scale
                nc.scalar.activation(gb_sb[:], gb_sb[:], AF.Tanh,
                                     scale=0.5 / WS)

                # aug = h + g*mo = (h + 0.5*mo) + (0.5*mo)*tanh
                u_sb = csb.tile([128, HID], F32, tag="u")
                nc.vector.scalar_tensor_tensor(
                    u_sb[:], mo_sb[:], 0.5, h_sb[:], op0=OP.mult, op1=OP.add)
                v_sb = csb.tile([128, HID], F32, tag="v")
                nc.vector.scalar_tensor_tensor(
                    v_sb[:], gb_sb[:], 0.5, mo_sb[:], op0=OP.mult, op1=OP.mult)
                nc.vector.scalar_tensor_tensor(
                    u_sb[:], u_sb[:], 0.0, v_sb[:], op0=OP.add, op1=OP.add,
                    accum_out=sum_all[:, t:t + 1])
                # square's tensor output is scrap; we only keep the accumulator
                nc.scalar.activation(
                    v_sb[:], u_sb[:], AF.Square, accum_out=ss_all[:, t:t + 1])

                # ---- LayerNorm finalize, per tile, VectorE only ----
                mean = stp.tile([128, 1], F32, tag="mean")
                nc.vector.tensor_scalar_mul(mean[:], sum_all[:, t:t + 1], 1.0 / HID)
                m2 = stp.tile([128, 1], F32, tag="m2")
                nc.vector.tensor_mul(m2[:], mean[:], mean[:])
                nc.vector.tensor_scalar_add(m2[:], m2[:], -LN_EPS)
                vpe = stp.tile([128, 1], F32, tag="vpe")
                nc.vector.scalar_tensor_tensor(
                    vpe[:], ss_all[:, t:t + 1], 1.0 / HID, m2[:],
                    op0=OP.mult, op1=OP.subtract)
                # rstd = 1/sqrt(vpe): quake init + 3 Newton iterations
                y = stp.tile([128, 1], F32, tag="y")
                yi = y[:].bitcast(I32)
                nc.vector.tensor_scalar(
                    yi, vpe[:].bitcast(I32), 1, None,
                    op0=OP.logical_shift_right)
                nc.vector.tensor_scalar(
                    yi, yi, -RSQRT_MAGIC, -1,
                    op0=OP.add, op1=OP.mult)
                yy = stp.tile([128, 1], F32, tag="yy")
                hw = stp.tile([128, 1], F32, tag="hw")
                for _ in range(3):
                    nc.vector.tensor_mul(yy[:], y[:], y[:])
                    nc.vector.tensor_mul(yy[:], yy[:], vpe[:])
                    nc.vector.tensor_scalar(
                        hw[:], yy[:], -0.5, 1.5, op0=OP.mult, op1=OP.add)
                    nc.vector.tensor_mul(y[:], y[:], hw[:])

                # yout = (aug - mean)*rstd*lng + lnb
                nc.vector.scalar_tensor_tensor(
                    u_sb[:], u_sb[:], mean[:], lng_sb[:],
                    op0=OP.subtract, op1=OP.mult)
                yo_sb = csb.tile([128, HID], F32, tag="yo")
                nc.vector.scalar_tensor_tensor(
                    yo_sb[:], u_sb[:], y[:], lnb_sb[:],
                    op0=OP.mult, op1=OP.add)
                nc.sync.dma_start(out_d.ap()[t], yo_sb[:])

        pAO_cm.__exit__(None, None, None)   # release attnout
        pWO_cm.__exit__(None, None, None)   # release Wo/Wg/Wog/eye

    nc.compile()
    return nc


def _prep_core(hs, mk, mv, nt):
    """Host-side lossless layout prep for one core's shard."""
    hT = np.ascontiguousarray(
        hs.reshape(nt, 128, NIC, 128).transpose(0, 3, 2, 1))      # [t,p,ic,b]
    h = np.ascontiguousarray(hs.reshape(nt, 128, HID))
    mkT = np.ascontiguousarray(
        mk.reshape(nt, 128, TOPK, NIC, 128).transpose(0, 2, 4, 3, 1))
    mvT = np.ascontiguousarray(
        mv.reshape(nt, 128, TOPK, NIC, 128).transpose(0, 2, 4, 3, 1))
    return hT, h, mkT, mvT


def kernel(**inputs):
    hs = np.asarray(inputs["hidden_state"], dtype=np.float32)
    mk = np.asarray(inputs["memory_keys"], dtype=np.float32)
    mv = np.asarray(inputs["memory_values"], dtype=np.float32)

    import ml_dtypes
    bf = ml_dtypes.bfloat16
    f8 = ml_dtypes.float8_e4m3

    def wlay(w):
        return np.ascontiguousarray(
            np.asarray(w, np.float32).reshape(NIC, 128, HID).transpose(1, 0, 2))

    wg_full = np.asarray(inputs["Wg"], np.float32)
    wo_f32 = np.asarray(inputs["Wo"], np.float32)
    wog_f32 = wo_f32 @ wg_full[HID:]          # fold Wo into the gate branch

    wq = wlay(inputs["Wq"]).astype(bf)
    wk = (wlay(inputs["Wk"]) * WS).astype(f8)
    wv = wlay(inputs["Wv"]).astype(bf)
    wo = wlay(wo_f32).astype(bf)
    wg1 = (wlay(wg_full[:HID]) * WS).astype(f8)
    wog = (wlay(wog_f32) * WS).astype(bf)
    bgb = np.ascontiguousarray(
        np.broadcast_to(np.asarray(inputs["bg"], np.float32) * WS, (128, HID)))
    lng = np.ascontiguousarray(
        np.broadcast_to(np.asarray(inputs["ln_g"], np.float32), (128, HID)))
    lnb = np.ascontiguousarray(
        np.broadcast_to(np.asarray(inputs["ln_b"], np.float32), (128, HID)))
    eye = np.eye(128, dtype=np.float32)

    if "nc" not in _CACHE:
        _CACHE["nc"] = _build(NT)
    nc = _CACHE["nc"]

    in_maps = []
    for c in range(N_CORES):
        sl = slice(c * BC, (c + 1) * BC)
        hT, h, mkT, mvT = _prep_core(hs[sl], mk[sl], mv[sl], NT)
        in_maps.append({
            "hTb": hT.astype(bf), "hT8": hT.astype(f8), "h": h,
            "mkT": mkT.astype(f8), "mvT": mvT.astype(bf),
            "Wq": wq, "Wk": wk, "Wv": wv, "Wo": wo, "Wg1": wg1, "Wog": wog,
            "bgB": bgb, "eye": eye, "lngB": lng, "lnbB": lnb,
        })

    res = run_bass_kernel_spmd(nc, in_maps, core_ids=list(range(N_CORES)),
                               trace=TRACE)
    kernel.last_result = res
    out = np.concatenate(
        [r["out"].reshape(BC, HID) for r in res.results], axis=0)
    return out


kernel.last_result = None


# revision 14
# speedup vs baseline: 1.2473x; 1.0390x over previous
"""Trainium2 Bass kernel for nn_EngramModule: single-query top-k memory attention
with gated residual + LayerNorm, data-parallel across 8 NeuronCores.

Contract: kernel(**inputs) takes the FULL unsharded inputs and returns the FULL
(8192, 1024) float32 output.

Per-core pipeline (1024 batch rows, 8 row-tiles of 128):
  A+B (merged, software-pipelined per tile): Q = h @ Wq in bf16, then per
      k-slot: Kp projection in fp8 e4m3 DoubleRow mode (256-row contraction
      per instruction, ~2x bf16 throughput; Wk pre-scaled by 32 on host to
      dodge e4m3 subnormals, folded back in the exp scale); Vp projection in
      bf16 (error budget: V feeds the output directly, the K path only
      perturbs softmax weights).  Kp is evicted PSUM->SBUF as bf16 by
      ScalarE so the score multiply+reduce run the DVE 2x packed mode;
      attn-weighted V accumulates in f32 with GpSimd adds (its only job).
  C:  memory_out = attnout @ Wo in bf16.  The gate's mo-branch is folded on
      host: mo @ Wg2 = ao @ (Wo @ Wg2) = ao @ Wog, so no mo transpose or
      second eviction chain exists; gate = hT8 @ Wg1 + aoT8 @ Wog, both in
      fp8 DoubleRow (weights *32, folded into the tanh scale).
      sigmoid(x) = 0.5*tanh(x/2)+0.5 keeps ScalarE in one ACT table set;
      aug = h + g*mo; LayerNorm finalizes per tile with a VectorE-only
      Newton rsqrt (bit-trick init).

Bulk weight streams ride the ScalarE HWDGE queue; latency-critical per-tile
loads ride the SyncE queue so they are never stuck behind megabyte weight
transfers. Activations are pre-laid-out on host (pure transpose/reshape,
lossless) so the contraction dim sits on SBUF partitions and no on-chip
transposes of the big tensors are needed.
"""

import os
import sys

import numpy as np

for _p in ("/opt/trn_rl_repo", "/root/.axon_site/_ro/trn_rl_repo"):
    if os.path.isdir(_p) and _p not in sys.path:
        sys.path.insert(0, _p)

from contextlib import ExitStack

import concourse.bacc as bacc
import concourse.mybir as mybir
import concourse.tile as tile
from concourse.bass_utils import run_bass_kernel_spmd

F32 = mybir.dt.float32
F32R = mybir.dt.float32r
BF16 = mybir.dt.bfloat16
F8 = mybir.dt.float8e4
I32 = mybir.dt.int32
AX = mybir.AxisListType
OP = mybir.AluOpType
AF = mybir.ActivationFunctionType
DR = mybir.MatmulPerfMode.DoubleRow

N_CORES = 8
B = 8192
HID = 1024
NH = 16
DH = 64
TOPK = 8
LN_EPS = 1e-5

BC = B // N_CORES          # rows per core = 1024
NT = BC // 128             # row-tiles per core = 8
NIC = HID // 128           # 128-row contraction chunks = 8
NICP = NIC // 2            # fp8 DoubleRow 256-row chunk pairs = 4
NJH = HID // 512           # 512-wide output halves = 2
SCALE = DH ** -0.5
WS = 32.0                  # host pre-scale on fp8 weights (power of 2)
RSQRT_MAGIC = 0x5F3759DF

# Set by test.py to collect a profile; grading path leaves this off.
TRACE = False

_CACHE = {}


def _build(nt=NT):
    nc = bacc.Bacc("TRN2", target_bir_lowering=False, debug=False,
                   num_devices=N_CORES)

    # ---- DRAM parameters (per-core shard, host-prepped layouts) ----
    h_d = nc.declare_dram_parameter("h", [nt, 128, HID], F32, isOutput=False)
    mkT_d = nc.declare_dram_parameter("mkT", [nt, TOPK, 128, NIC, 128], F8, isOutput=False)
    mvT_d = nc.declare_dram_parameter("mvT", [nt, TOPK, 128, NIC, 128], BF16, isOutput=False)
    hTb_d = nc.declare_dram_parameter("hTb", [nt, 128, NIC, 128], BF16, isOutput=False)
    hT8_d = nc.declare_dram_parameter("hT8", [nt, 128, NIC, 128], F8, isOutput=False)
    wq_d = nc.declare_dram_parameter("Wq", [128, NIC, HID], BF16, isOutput=False)
    wk_d = nc.declare_dram_parameter("Wk", [128, NIC, HID], F8, isOutput=False)
    wv_d = nc.declare_dram_parameter("Wv", [128, NIC, HID], BF16, isOutput=False)
    wo_d = nc.declare_dram_parameter("Wo", [128, NIC, HID], BF16, isOutput=False)
    wg1_d = nc.declare_dram_parameter("Wg1", [128, NIC, HID], F8, isOutput=False)
    wog_d = nc.declare_dram_parameter("Wog", [128, NIC, HID], BF16, isOutput=False)
    bgb_d = nc.declare_dram_parameter("bgB", [128, HID], F32, isOutput=False)
    eye_d = nc.declare_dram_parameter("eye", [128, 128], F32, isOutput=False)
    lng_d = nc.declare_dram_parameter("lngB", [128, HID], F32, isOutput=False)
    lnb_d = nc.declare_dram_parameter("lnbB", [128, HID], F32, isOutput=False)
    out_d = nc.declare_dram_parameter("out", [nt, 128, HID], F32, isOutput=True)

    def load_w(tile_sb, dram, nchunk):
        # bulk weights on the ScalarE HWDGE queue, chunked so the first
        # dependent matmul only waits for its own chunk
        for ic in range(nchunk):
            nc.scalar.dma_start(tile_sb[:, ic, :], dram.ap()[:, ic, :])

    with ExitStack() as octx:
        tc = octx.enter_context(tile.TileContext(nc))

        pers = octx.enter_context(tc.tile_pool(name="pers", bufs=1))
        sum_all = pers.tile([128, nt], F32, tag="sum_all")
        ss_all = pers.tile([128, nt], F32, tag="ss_all")

        # long-lived weights / cross-phase activations
        pWO_cm = tc.tile_pool(name="pWO", bufs=1); pWO = pWO_cm.__enter__()
        wo_sb = pWO.tile([128, NIC, HID], BF16, tag="wo")
        wg1_sb = pWO.tile([128, NIC, HID], F8, tag="wg1")
        wog_sb = pWO.tile([128, NIC, HID], BF16, tag="wog")
        eye_sb = pWO.tile([128, 128], F32, tag="eye")

        pAO_cm = tc.tile_pool(name="pAO", bufs=1); pAO = pAO_cm.__enter__()
        ao_all = pAO.tile([128, nt, HID], F32, tag="ao_all")

        pAB_cm = tc.tile_pool(name="pAB", bufs=1); pAB = pAB_cm.__enter__()
        q_all = pAB.tile([128, nt, HID], BF16, tag="q_all")

        pWKV_cm = tc.tile_pool(name="wkv", bufs=1); wkv = pWKV_cm.__enter__()
        wq_sb = wkv.tile([128, NIC, HID], BF16, tag="wq")
        wk_sb = wkv.tile([128, NIC, HID], F8, tag="wk")
        wv_sb = wkv.tile([128, NIC, HID], BF16, tag="wv")

        # ============ merged phase A+B: Q projection + attention ============
        with ExitStack() as bctx:
            hp = bctx.enter_context(tc.tile_pool(name="hT_a", bufs=1))
            mp = bctx.enter_context(tc.tile_pool(name="mkv", bufs=4))
            kvps = bctx.enter_context(tc.tile_pool(name="kv_ps", bufs=2, space="PSUM"))
            sp = bctx.enter_context(tc.tile_pool(name="scr", bufs=2))
            accp = bctx.enter_context(tc.tile_pool(name="acc", bufs=2))
            ep = bctx.enter_context(tc.tile_pool(name="e", bufs=2))

            # scalar HWDGE queue: only the weights phase B needs right away,
            # chunked so the first dependent matmul waits only for its chunk.
            # Everything consumed later rides the otherwise-idle GpSimd
            # software-DGE queue as whole-tile transfers, keeping the ScalarE
            # instruction stream free for the loop's evictions/exps.
            load_w(wq_sb, wq_d, NIC)
            load_w(wk_sb, wk_d, NIC)
            load_w(wv_sb, wv_d, NIC)
            nc.gpsimd.dma_start(wo_sb[:], wo_d.ap())
            nc.gpsimd.dma_start(wg1_sb[:], wg1_d.ap())
            nc.gpsimd.dma_start(wog_sb[:], wog_d.ap())
            nc.gpsimd.dma_start(eye_sb[:], eye_d.ap())

            # resident hT (bf16, for Q); sync queue ahead of the mk/mv stream
            hT_all = hp.tile([128, nt, NIC, 128], BF16, tag="hT")
            for t in range(nt):
                nc.sync.dma_start(hT_all[:, t], hTb_d.ap()[t])
            preload = {}
            for (pt, pk) in ((0, 0),):
                a = mp.tile([128, NIC, 128], F8, tag="mkT")
                nc.sync.dma_start(a[:], mkT_d.ap()[pt, pk])
                b_ = mp.tile([128, NIC, 128], BF16, tag="mvT")
                nc.sync.dma_start(b_[:], mvT_d.ap()[pt, pk])
                preload[(pt, pk)] = (a, b_)

            for t in range(nt):
                # ---- Q projection for this tile (PSUM slot shared w/ kp) ----
                q_ps = kvps.tile([128, HID], F32, tag="kp")
                for ic in range(NIC):
                    for jh in range(NJH):
                        nc.tensor.matmul(
                            q_ps[:, jh * 512:(jh + 1) * 512],
                            hT_all[:, t, ic, :],
                            wq_sb[:, ic, jh * 512:(jh + 1) * 512],
                            start=(ic == 0), stop=(ic == NIC - 1),
                        )
                nc.scalar.copy(q_all[:, t, :], q_ps[:])

                acc = accp.tile([128, HID], BF16, tag="acc")
                e_all = ep.tile([128, TOPK, NH], BF16, tag="e_all")
                q_t = q_all[:, t, :]
                for k in range(TOPK):
                    if (t, k) in preload:
                        mkT, mvT = preload[(t, k)]
                    else:
                        mkT = mp.tile([128, NIC, 128], F8, tag="mkT")
                        nc.sync.dma_start(mkT[:], mkT_d.ap()[t, k])
                        mvT = mp.tile([128, NIC, 128], BF16, tag="mvT")
                        nc.sync.dma_start(mvT[:], mvT_d.ap()[t, k])

                    kp_ps = kvps.tile([128, HID], F32, tag="kp")
                    vp_ps = kvps.tile([128, HID], F32, tag="vp")
                    # K projection: fp8 DoubleRow, 256-row contraction pairs
                    for ip in range(NICP):
                        for jh in range(NJH):
                            nc.tensor.matmul(
                                kp_ps[:, jh * 512:(jh + 1) * 512],
                                mkT[:, 2 * ip:2 * ip + 2, :],
                                wk_sb[:, 2 * ip:2 * ip + 2, jh * 512:(jh + 1) * 512],
                                start=(ip == 0), stop=(ip == NICP - 1),
                                perf_mode=DR,
                            )
                    for ic in range(NIC):
                        for jh in range(NJH):
                            nc.tensor.matmul(
                                vp_ps[:, jh * 512:(jh + 1) * 512],
                                mvT[:, ic, :],
                                wv_sb[:, ic, jh * 512:(jh + 1) * 512],
                                start=(ic == 0), stop=(ic == NIC - 1),
                            )

                    # scores for all 16 heads of this k-slot (bf16 products
                    # so the per-head reduce reads packed 16-bit)
                    p_scr = sp.tile([128, HID], BF16, tag="p")
                    nc.vector.tensor_mul(p_scr[:], q_t, kp_ps[:])
                    s_k = ep.tile([128, NH], F32, tag="s_k")
                    nc.vector.reduce_sum(
                        s_k[:], p_scr[:].rearrange("p (h d) -> p h d", h=NH), axis=AX.X)
                    # e = exp(scores * DH**-0.5 / WS); logits ~N(0,1) so no
                    # max-sub; /WS folds out the host Wk pre-scale
                    nc.scalar.activation(e_all[:, k, :], s_k[:], AF.Exp,
                                         scale=SCALE / WS)

                    # weighted V accumulate: acc += e[:,k,h] (bcast over d) * Vp
                    e_bc = e_all[:, k, :].unsqueeze(2).broadcast_to([128, NH, DH])
                    dst = acc if k == 0 else sp.tile([128, HID], BF16, tag="pv")
                    nc.vector.tensor_tensor(
                        dst[:].rearrange("p (h d) -> p h d", h=NH),
                        vp_ps[:].rearrange("p (h d) -> p h d", h=NH),
                        e_bc, op=OP.mult)
                    if k > 0:
                        nc.vector.tensor_add(acc[:], acc[:], dst[:])

                # normalize: attnout = acc * (1/sum_k e)
                den = ep.tile([128, NH], F32, tag="den")
                nc.vector.reduce_sum(
                    den[:], e_all[:].rearrange("p k h -> p h k"), axis=AX.X)
                rden = ep.tile([128, NH], F32, tag="rden")
                nc.vector.reciprocal(rden[:], den[:])
                rden_bc = rden[:].unsqueeze(2).broadcast_to([128, NH, DH])
                nc.vector.tensor_tensor(
                    ao_all[:, t, :].rearrange("p (h d) -> p h d", h=NH),
                    acc[:].rearrange("p (h d) -> p h d", h=NH),
                    rden_bc, op=OP.mult)

        pWKV_cm.__exit__(None, None, None)  # release Wq/Wk/Wv
        pAB_cm.__exit__(None, None, None)   # release q_all

        # ===== phase C: memory_out, gate, residual, LayerNorm, output =====
        with ExitStack() as cctx:
            eyep = cctx.enter_context(tc.tile_pool(name="cconst", bufs=1))
            cstr = cctx.enter_context(tc.tile_pool(name="c_str", bufs=2))
            csb = cctx.enter_context(tc.tile_pool(name="c_sb", bufs=2))
            stp = cctx.enter_context(tc.tile_pool(name="stats", bufs=2))
            tps = cctx.enter_context(tc.tile_pool(name="tp_ps", bufs=1, space="PSUM"))
            mps = cctx.enter_context(tc.tile_pool(name="mo_ps", bufs=1, space="PSUM"))
            gps = cctx.enter_context(tc.tile_pool(name="g_ps", bufs=2, space="PSUM"))

            # epilogue constants on the GpSimd queue (idle during B/C)
            bgb_sb = eyep.tile([128, HID], F32, tag="bgb")
            nc.gpsimd.dma_start(bgb_sb[:], bgb_d.ap())
            lng_sb = eyep.tile([128, HID], F32, tag="lng")
            nc.gpsimd.dma_start(lng_sb[:], lng_d.ap())
            lnb_sb = eyep.tile([128, HID], F32, tag="lnb")
            nc.gpsimd.dma_start(lnb_sb[:], lnb_d.ap())

            for t in range(nt):
                hT_sb = cstr.tile([128, NIC, 128], F8, tag="hT_c")
                nc.gpsimd.dma_start(hT_sb[:], hT8_d.ap()[t])
                h_sb = cstr.tile([128, HID], F32, tag="h_c")
                nc.gpsimd.dma_start(h_sb[:], h_d.ap()[t])

                # attn-out transposes, then both evictions (bf16 for Wo,
                # fp8 for the gate's Wog branch)
                at_ps = tps.tile([128, NIC, 128], F32, tag="tp_ps")
                for ic in range(NIC):
                    nc.tensor.transpose(
                        at_ps[:, ic, :], ao_all[:, t, ic * 128:(ic + 1) * 128],
                        eye_sb[:])
                atT_sb = csb.tile([128, NIC, 128], BF16, tag="atT")
                nc.scalar.copy(atT_sb[:], at_ps[:])

                # gate h-branch: fp8 DoubleRow; fills TensorE while the
                # evictions drain on ScalarE
                g_ps = gps.tile([128, HID], F32, tag="g_ps")
                for ip in range(NICP):
                    for jh in range(NJH):
                        sl = slice(jh * 512, (jh + 1) * 512)
                        nc.tensor.matmul(
                            g_ps[:, sl], hT_sb[:, 2 * ip:2 * ip + 2, :],
                            wg1_sb[:, 2 * ip:2 * ip + 2, sl],
                            start=(ip == 0), stop=False, perf_mode=DR)

                mo_ps = mps.tile([128, HID], F32, tag="mo_ps")
                for ic in range(NIC):
                    for jh in range(NJH):
                        nc.tensor.matmul(
                            mo_ps[:, jh * 512:(jh + 1) * 512],
                            atT_sb[:, ic, :],
                            wo_sb[:, ic, jh * 512:(jh + 1) * 512],
                            start=(ic == 0), stop=(ic == NIC - 1),
                        )

                # gate ao-branch: ao @ (Wo @ Wg2) folded on host (bf16: fp8
                # noise here spikes the max-err tail, and the fold already
                # removed the transpose chain)
                for ic in range(NIC):
                    for jh in range(NJH):
                        sl = slice(jh * 512, (jh + 1) * 512)
                        nc.tensor.matmul(
                            g_ps[:, sl], atT_sb[:, ic, :],
                            wog_sb[:, ic, sl],
                            start=False, stop=(ic == NIC - 1))

                mo_sb = csb.tile([128, HID], F32, tag="mo")
                nc.scalar.copy(mo_sb[:], mo_ps[:])

                gb_sb = csb.tile([128, HID], F32, tag="gb")
                nc.vector.tensor_add(gb_sb[:], g_ps[:], bgb_sb[:])
                # sigmoid(x) = 0.5*tanh(x/2) + 0.5 (tanh shares ACT set w/ exp)
                # scale folds out the host Wg pre-scale
                nc.scalar.activation(gb_sb[:], gb_sb[:], AF.Tanh,
                                     scale=0.5 / WS)

                # aug = h + g*mo = (h + 0.5*mo) + (0.5*mo)*tanh
                u_sb = csb.tile([128, HID], F32, tag="u")
                nc.vector.scalar_tensor_tensor(
                    u_sb[:], mo_sb[:], 0.5, h_sb[:], op0=OP.mult, op1=OP.add)
                v_sb = csb.tile([128, HID], F32, tag="v")
                nc.vector.scalar_tensor_tensor(
                    v_sb[:], gb_sb[:], 0.5, mo_sb[:], op0=OP.mult, op1=OP.mult)
                nc.vector.scalar_tensor_tensor(
                    u_sb[:], u_sb[:], 0.0, v_sb[:], op0=OP.add, op1=OP.add,
                    accum_out=sum_all[:, t:t + 1])
                # square's tensor output is scrap; we only keep the accumulator
                nc.scalar.activation(
                    v_sb[:], u_sb[:], AF.Square, accum_out=ss_all[:, t:t + 1])

                # ---- LayerNorm finalize, per tile, VectorE only ----
                mean = stp.tile([128, 1], F32, tag="mean")
                nc.vector.tensor_scalar_mul(mean[:], sum_all[:, t:t + 1], 1.0 / HID)
                m2 = stp.tile([128, 1], F32, tag="m2")
                nc.vector.tensor_mul(m2[:], mean[:], mean[:])
                nc.vector.tensor_scalar_add(m2[:], m2[:], -LN_EPS)
                vpe = stp.tile([128, 1], F32, tag="vpe")
                nc.vector.scalar_tensor_tensor(
                    vpe[:], ss_all[:, t:t + 1], 1.0 / HID, m2[:],
                    op0=OP.mult, op1=OP.subtract)
                # rstd = 1/sqrt(vpe): quake init + 3 Newton iterations
                y = stp.tile([128, 1], F32, tag="y")
                yi = y[:].bitcast(I32)
                nc.vector.tensor_scalar(
                    yi, vpe[:].bitcast(I32), 1, None,
                    op0=OP.logical_shift_right)
                nc.vector.tensor_scalar(
                    yi, yi, -RSQRT_MAGIC, -1,
                    op0=OP.add, op1=OP.mult)
                yy = stp.tile([128, 1], F32, tag="yy")
                hw = stp.tile([128, 1], F32, tag="hw")
                for _ in range(3):
                    nc.vector.tensor_mul(yy[:], y[:], y[:])
                    nc.vector.tensor_mul(yy[:], yy[:], vpe[:])
                    nc.vector.tensor_scalar(
                        hw[:], yy[:], -0.5, 1.5, op0=OP.mult, op1=OP.add)
                    nc.vector.tensor_mul(y[:], y[:], hw[:])

                # yout = (aug - mean)*rstd*lng + lnb
                nc.vector.scalar_tensor_tensor(
                    u_sb[:], u_sb[:], mean[:], lng_sb[:],
                    op0=OP.subtract, op1=OP.mult)
                yo_sb = csb.tile([128, HID], F32, tag="yo")
                nc.vector.scalar_tensor_tensor(
                    yo_sb[:], u_sb[:], y[:], lnb_sb[:],
                    op0=OP.mult, op1=OP.add)
                nc.sync.dma_start(out_d.ap()[t], yo_sb[:])

        pAO_cm.__exit__(None, None, None)   # release attnout
        pWO_cm.__exit__(None, None, None)   # release Wo/Wg/Wog/eye

    nc.compile()
    return nc


def _prep_core(hs, mk, mv, nt):
    """Host-side lossless layout prep for one core's shard."""
    hT = np.ascontiguousarray(
        hs.reshape(nt, 128, NIC, 128).transpose(0, 3, 2, 1))      # [t,p,ic,b]
    h = np.ascontiguousarray(hs.reshape(nt, 128, HID))
    mkT = np.ascontiguousarray(
        mk.reshape(nt, 128, TOPK, NIC, 128).transpose(0, 2, 4, 3, 1))
    mvT = np.ascontiguousarray(
        mv.reshape(nt, 128, TOPK, NIC, 128).transpose(0, 2, 4, 3, 1))
    return hT, h, mkT, mvT


def kernel(**inputs):
    hs = np.asarray(inputs["hidden_state"], dtype=np.float32)
    mk = np.asarray(inputs["memory_keys"], dtype=np.float32)
    mv = np.asarray(inputs["memory_values"], dtype=np.float32)

    import ml_dtypes
    bf = ml_dtypes.bfloat16
    f8 = ml_dtypes.float8_e4m3

    def wlay(w):
        return np.ascontiguousarray(
            np.asarray(w, np.float32).reshape(NIC, 128, HID).transpose(1, 0, 2))

    wg_full = np.asarray(inputs["Wg"], np.float32)
    wo_f32 = np.asarray(inputs["Wo"], np.float32)
    wog_f32 = wo_f32 @ wg_full[HID:]          # fold Wo into the gate branch

    wq = wlay(inputs["Wq"]).astype(bf)
    wk = (wlay(inputs["Wk"]) * WS).astype(f8)
    wv = wlay(inputs["Wv"]).astype(bf)
    wo = wlay(wo_f32).astype(bf)
    wg1 = (wlay(wg_full[:HID]) * WS).astype(f8)
    wog = (wlay(wog_f32) * WS).astype(bf)
    bgb = np.ascontiguousarray(
        np.broadcast_to(np.asarray(inputs["bg"], np.float32) * WS, (128, HID)))
    lng = np.ascontiguousarray(
        np.broadcast_to(np.asarray(inputs["ln_g"], np.float32), (128, HID)))
    lnb = np.ascontiguousarray(
        np.broadcast_to(np.asarray(inputs["ln_b"], np.float32), (128, HID)))
    eye = np.eye(128, dtype=np.float32)

    if "nc" not in _CACHE:
        _CACHE["nc"] = _build(NT)
    nc = _CACHE["nc"]

    in_maps = []
    for c in range(N_CORES):
        sl = slice(c * BC, (c + 1) * BC)
        hT, h, mkT, mvT = _prep_core(hs[sl], mk[sl], mv[sl], NT)
        in_maps.append({
            "hTb": hT.astype(bf), "hT8": hT.astype(f8), "h": h,
            "mkT": mkT.astype(f8), "mvT": mvT.astype(bf),
            "Wq": wq, "Wk": wk, "Wv": wv, "Wo": wo, "Wg1": wg1, "Wog": wog,
            "bgB": bgb, "eye": eye, "lngB": lng, "lnbB": lnb,
        })

    res = run_bass_kernel_spmd(nc, in_maps, core_ids=list(range(N_CORES)),
                               trace=TRACE)
    kernel.last_result = res
    out = np.concatenate(
        [r["out"].reshape(BC, HID) for r in res.results], axis=0)
    return out


kernel.last_result = None


# revision 26
# speedup vs baseline: 1.2883x; 1.0329x over previous
"""Trainium2 Bass kernel for nn_EngramModule: single-query top-k memory attention
with gated residual + LayerNorm, data-parallel across 8 NeuronCores.

Contract: kernel(**inputs) takes the FULL unsharded inputs and returns the FULL
(8192, 1024) float32 output.

Per-core pipeline (1024 batch rows, 8 row-tiles of 128):
  A+B (merged, software-pipelined per tile): Q = h @ Wq in bf16, then per
      k-slot: Kp projection in fp8 e4m3 DoubleRow mode (256-row contraction
      per instruction, ~2x bf16 throughput; Wk pre-scaled by 32 on host to
      dodge e4m3 subnormals, folded back in the exp scale); Vp projection in
      bf16 (error budget: V feeds the output directly, the K path only
      perturbs softmax weights).  Kp is evicted PSUM->SBUF as bf16 by
      ScalarE so the score multiply+reduce run the DVE 2x packed mode;
      attn-weighted V accumulates in f32 with GpSimd adds (its only job).
  C:  memory_out = attnout @ Wo in bf16.  The gate's mo-branch is folded on
      host: mo @ Wg2 = ao @ (Wo @ Wg2) = ao @ Wog, so no mo transpose or
      second eviction chain exists; gate = hT8 @ Wg1 + aoT8 @ Wog, both in
      fp8 DoubleRow (weights *32, folded into the tanh scale).
      sigmoid(x) = 0.5*tanh(x/2)+0.5 keeps ScalarE in one ACT table set;
      aug = h + g*mo; LayerNorm finalizes per tile with a VectorE-only
      Newton rsqrt (bit-trick init).

Bulk weight streams ride the ScalarE HWDGE queue; latency-critical per-tile
loads ride the SyncE queue so they are never stuck behind megabyte weight
transfers. Activations are pre-laid-out on host (pure transpose/reshape,
lossless) so the contraction dim sits on SBUF partitions and no on-chip
transposes of the big tensors are needed.
"""

import os
import sys

import numpy as np

for _p in ("/opt/trn_rl_repo", "/root/.axon_site/_ro/trn_rl_repo"):
    if os.path.isdir(_p) and _p not in sys.path:
        sys.path.insert(0, _p)

from contextlib import ExitStack

import concourse.bacc as bacc
import concourse.mybir as mybir
import concourse.tile as tile
from concourse.bass_utils import run_bass_kernel_spmd

F32 = mybir.dt.float32
F32R = mybir.dt.float32r
BF16 = mybir.dt.bfloat16
F8 = mybir.dt.float8e4
I32 = mybir.dt.int32
AX = mybir.AxisListType
OP = mybir.AluOpType
AF = mybir.ActivationFunctionType
DR = mybir.MatmulPerfMode.DoubleRow

N_CORES = 8
B = 8192
HID = 1024
NH = 16
DH = 64
TOPK = 8
LN_EPS = 1e-5

BC = B // N_CORES          # rows per core = 1024
NT = BC // 128             # row-tiles per core = 8
NIC = HID // 128           # 128-row contraction chunks = 8
NICP = NIC // 2            # fp8 DoubleRow 256-row chunk pairs = 4
NJH = HID // 512           # 512-wide output halves = 2
SCALE = DH ** -0.5
WS = 32.0                  # host pre-scale on fp8 weights (power of 2)
RSQRT_MAGIC = 0x5F3759DF

# Set by test.py to collect a profile; grading path leaves this off.
TRACE = False

_CACHE = {}


def _build(nt=NT):
    nc = bacc.Bacc("TRN2", target_bir_lowering=False, debug=False,
                   num_devices=N_CORES)

    # ---- DRAM parameters (per-core shard, host-prepped layouts) ----
    h_d = nc.declare_dram_parameter("h", [nt, 128, HID], F32, isOutput=False)
    mkT_d = nc.declare_dram_parameter("mkT", [nt, TOPK, 128, NIC, 128], F8, isOutput=False)
    mvT_d = nc.declare_dram_parameter("mvT", [nt, TOPK, 128, NIC, 128], BF16, isOutput=False)
    hT8_d = nc.declare_dram_parameter("hT8", [nt, 128, NIC, 128], F8, isOutput=False)
    wq_d = nc.declare_dram_parameter("Wq", [128, NIC, HID], F8, isOutput=False)
    wk_d = nc.declare_dram_parameter("Wk", [128, NIC, HID], F8, isOutput=False)
    wv_d = nc.declare_dram_parameter("Wv", [128, NIC, HID], BF16, isOutput=False)
    wo_d = nc.declare_dram_parameter("Wo", [128, NIC, HID], BF16, isOutput=False)
    wg1_d = nc.declare_dram_parameter("Wg1", [128, NIC, HID], F8, isOutput=False)
    wog_d = nc.declare_dram_parameter("Wog", [128, NIC, HID], BF16, isOutput=False)
    bgb_d = nc.declare_dram_parameter("bgB", [128, HID], F32, isOutput=False)
    eye_d = nc.declare_dram_parameter("eye", [128, 128], F32, isOutput=False)
    lng_d = nc.declare_dram_parameter("lngB", [128, HID], F32, isOutput=False)
    lnb_d = nc.declare_dram_parameter("lnbB", [128, HID], F32, isOutput=False)
    out_d = nc.declare_dram_parameter("out", [nt, 128, HID], F32, isOutput=True)

    def load_w(tile_sb, dram, nchunk):
        # bulk weights on the ScalarE HWDGE queue, chunked so the first
        # dependent matmul only waits for its own chunk
        for ic in range(nchunk):
            nc.scalar.dma_start(tile_sb[:, ic, :], dram.ap()[:, ic, :])

    with ExitStack() as octx:
        tc = octx.enter_context(tile.TileContext(nc))

        pers = octx.enter_context(tc.tile_pool(name="pers", bufs=1))
        sum_all = pers.tile([128, nt], F32, tag="sum_all")
        ss_all = pers.tile([128, nt], F32, tag="ss_all")

        # long-lived weights / cross-phase activations
        pWO_cm = tc.tile_pool(name="pWO", bufs=1); pWO = pWO_cm.__enter__()
        wo_sb = pWO.tile([128, NIC, HID], BF16, tag="wo")
        wg1_sb = pWO.tile([128, NIC, HID], F8, tag="wg1")
        wog_sb = pWO.tile([128, NIC, HID], BF16, tag="wog")
        eye_sb = pWO.tile([128, 128], F32, tag="eye")

        pAO_cm = tc.tile_pool(name="pAO", bufs=1); pAO = pAO_cm.__enter__()
        ao_all = pAO.tile([128, nt, HID], F32, tag="ao_all")

        # resident fp8 hidden-state transpose: Q projection (A+B) AND the
        # gate h-branch (C) both consume it
        pHT_cm = tc.tile_pool(name="pHT", bufs=1); pHT = pHT_cm.__enter__()
        hT_all = pHT.tile([128, nt, NIC, 128], F8, tag="hT")

        pAB_cm = tc.tile_pool(name="pAB", bufs=1); pAB = pAB_cm.__enter__()
        q_all = pAB.tile([128, nt, HID], BF16, tag="q_all")

        pWKV_cm = tc.tile_pool(name="wkv", bufs=1); wkv = pWKV_cm.__enter__()
        wq_sb = wkv.tile([128, NIC, HID], F8, tag="wq")
        wk_sb = wkv.tile([128, NIC, HID], F8, tag="wk")
        wv_sb = wkv.tile([128, NIC, HID], BF16, tag="wv")

        # ============ merged phase A+B: Q projection + attention ============
        with ExitStack() as bctx:
            mp = bctx.enter_context(tc.tile_pool(name="mkv", bufs=4))
            kvps = bctx.enter_context(tc.tile_pool(name="kv_ps", bufs=2, space="PSUM"))
            sp = bctx.enter_context(tc.tile_pool(name="scr", bufs=2))
            accp = bctx.enter_context(tc.tile_pool(name="acc", bufs=2))
            ep = bctx.enter_context(tc.tile_pool(name="e", bufs=2))

            # startup loads fan out over all three DMA queues so the first
            # tile isn't serialized behind one queue: scalar takes wq/wk
            # (chunked, consumed first), gpsimd (software DGE, otherwise
            # idle) takes wv + everything phase C needs, sync takes hT plus
            # the mkT/mvT stream.
            load_w(wq_sb, wq_d, NIC)
            load_w(wk_sb, wk_d, NIC)
            nc.gpsimd.dma_start(wv_sb[:], wv_d.ap())
            nc.gpsimd.dma_start(wo_sb[:], wo_d.ap())
            nc.gpsimd.dma_start(wg1_sb[:], wg1_d.ap())
            nc.gpsimd.dma_start(wog_sb[:], wog_d.ap())
            nc.gpsimd.dma_start(eye_sb[:], eye_d.ap())

            for t in range(nt):
                nc.sync.dma_start(hT_all[:, t], hT8_d.ap()[t])
            preload = {}
            for (pt, pk) in ((0, 0),):
                a = mp.tile([128, NIC, 128], F8, tag="mkT")
                nc.sync.dma_start(a[:], mkT_d.ap()[pt, pk])
                b_ = mp.tile([128, NIC, 128], BF16, tag="mvT")
                nc.sync.dma_start(b_[:], mvT_d.ap()[pt, pk])
                preload[(pt, pk)] = (a, b_)

            for t in range(nt):
                # ---- Q projection (fp8 DoubleRow; PSUM slot shared w/ kp) ----
                q_ps = kvps.tile([128, HID], F32, tag="kp")
                for ip in range(NICP):
                    for jh in range(NJH):
                        nc.tensor.matmul(
                            q_ps[:, jh * 512:(jh + 1) * 512],
                            hT_all[:, t, 2 * ip:2 * ip + 2, :],
                            wq_sb[:, 2 * ip:2 * ip + 2, jh * 512:(jh + 1) * 512],
                            start=(ip == 0), stop=(ip == NICP - 1),
                            perf_mode=DR,
                        )
                nc.scalar.copy(q_all[:, t, :], q_ps[:])

                acc = accp.tile([128, HID], BF16, tag="acc")
                e_all = ep.tile([128, TOPK, NH], BF16, tag="e_all")
                q_t = q_all[:, t, :]
                for k in range(TOPK):
                    if (t, k) in preload:
                        mkT, mvT = preload[(t, k)]
                    else:
                        mkT = mp.tile([128, NIC, 128], F8, tag="mkT")
                        nc.sync.dma_start(mkT[:], mkT_d.ap()[t, k])
                        mvT = mp.tile([128, NIC, 128], BF16, tag="mvT")
                        nc.sync.dma_start(mvT[:], mvT_d.ap()[t, k])

                    kp_ps = kvps.tile([128, HID], F32, tag="kp")
                    vp_ps = kvps.tile([128, HID], F32, tag="vp")
                    # K projection: fp8 DoubleRow, 256-row contraction pairs
                    for ip in range(NICP):
                        for jh in range(NJH):
                            nc.tensor.matmul(
                                kp_ps[:, jh * 512:(jh + 1) * 512],
                                mkT[:, 2 * ip:2 * ip + 2, :],
                                wk_sb[:, 2 * ip:2 * ip + 2, jh * 512:(jh + 1) * 512],
                                start=(ip == 0), stop=(ip == NICP - 1),
                                perf_mode=DR,
                            )
                    for ic in range(NIC):
                        for jh in range(NJH):
                            nc.tensor.matmul(
                                vp_ps[:, jh * 512:(jh + 1) * 512],
                                mvT[:, ic, :],
                                wv_sb[:, ic, jh * 512:(jh + 1) * 512],
                                start=(ic == 0), stop=(ic == NIC - 1),
                            )

                    # scores for all 16 heads of this k-slot (bf16 products
                    # so the per-head reduce reads packed 16-bit)
                    p_scr = sp.tile([128, HID], BF16, tag="p")
                    nc.vector.tensor_mul(p_scr[:], q_t, kp_ps[:])
                    s_k = ep.tile([128, NH], F32, tag="s_k")
                    nc.vector.reduce_sum(
                        s_k[:], p_scr[:].rearrange("p (h d) -> p h d", h=NH), axis=AX.X)
                    # e = exp(scores * DH**-0.5 / WS^2); logits ~N(0,1) so no
                    # max-sub; /WS^2 folds out the host Wq and Wk pre-scales
                    nc.scalar.activation(e_all[:, k, :], s_k[:], AF.Exp,
                                         scale=SCALE / (WS * WS))

                    # weighted V accumulate: acc += e[:,k,h] (bcast over d) * Vp
                    e_bc = e_all[:, k, :].unsqueeze(2).broadcast_to([128, NH, DH])
                    dst = acc if k == 0 else sp.tile([128, HID], BF16, tag="pv")
                    nc.vector.tensor_tensor(
                        dst[:].rearrange("p (h d) -> p h d", h=NH),
                        vp_ps[:].rearrange("p (h d) -> p h d", h=NH),
                        e_bc, op=OP.mult)
                    if k > 0:
                        nc.vector.tensor_add(acc[:], acc[:], dst[:])

                # normalize: attnout = acc * (1/sum_k e)
                den = ep.tile([128, NH], F32, tag="den")
                nc.vector.reduce_sum(
                    den[:], e_all[:].rearrange("p k h -> p h k"), axis=AX.X)
                rden = ep.tile([128, NH], F32, tag="rden")
                nc.vector.reciprocal(rden[:], den[:])
                rden_bc = rden[:].unsqueeze(2).broadcast_to([128, NH, DH])
                nc.vector.tensor_tensor(
                    ao_all[:, t, :].rearrange("p (h d) -> p h d", h=NH),
                    acc[:].rearrange("p (h d) -> p h d", h=NH),
                    rden_bc, op=OP.mult)

        pWKV_cm.__exit__(None, None, None)  # release Wq/Wk/Wv
        pAB_cm.__exit__(None, None, None)   # release q_all

        # ===== phase C: memory_out, gate, residual, LayerNorm, output =====
        with ExitStack() as cctx:
            eyep = cctx.enter_context(tc.tile_pool(name="cconst", bufs=1))
            cstr = cctx.enter_context(tc.tile_pool(name="c_str", bufs=2))
            csb = cctx.enter_context(tc.tile_pool(name="c_sb", bufs=2))
            stp = cctx.enter_context(tc.tile_pool(name="stats", bufs=2))
            tps = cctx.enter_context(tc.tile_pool(name="tp_ps", bufs=1, space="PSUM"))
            mps = cctx.enter_context(tc.tile_pool(name="mo_ps", bufs=1, space="PSUM"))
            gps = cctx.enter_context(tc.tile_pool(name="g_ps", bufs=2, space="PSUM"))

            # epilogue constants on the GpSimd queue (idle during B/C)
            bgb_sb = eyep.tile([128, HID], F32, tag="bgb")
            nc.gpsimd.dma_start(bgb_sb[:], bgb_d.ap())
            lng_sb = eyep.tile([128, HID], F32, tag="lng")
            nc.gpsimd.dma_start(lng_sb[:], lng_d.ap())
            lnb_sb = eyep.tile([128, HID], F32, tag="lnb")
            nc.gpsimd.dma_start(lnb_sb[:], lnb_d.ap())

            for t in range(nt):
                h_sb = cstr.tile([128, HID], F32, tag="h_c")
                nc.gpsimd.dma_start(h_sb[:], h_d.ap()[t])

                # attn-out transposes, then both evictions (bf16 for Wo,
                # fp8 for the gate's Wog branch)
                at_ps = tps.tile([128, NIC, 128], F32, tag="tp_ps")
                for ic in range(NIC):
                    nc.tensor.transpose(
                        at_ps[:, ic, :], ao_all[:, t, ic * 128:(ic + 1) * 128],
                        eye_sb[:])
                atT_sb = csb.tile([128, NIC, 128], BF16, tag="atT")
                nc.scalar.copy(atT_sb[:], at_ps[:])

                # gate h-branch: fp8 DoubleRow; fills TensorE while the
                # evictions drain on ScalarE
                g_ps = gps.tile([128, HID], F32, tag="g_ps")
                for ip in range(NICP):
                    for jh in range(NJH):
                        sl = slice(jh * 512, (jh + 1) * 512)
                        nc.tensor.matmul(
                            g_ps[:, sl], hT_all[:, t, 2 * ip:2 * ip + 2, :],
                            wg1_sb[:, 2 * ip:2 * ip + 2, sl],
                            start=(ip == 0), stop=False, perf_mode=DR)

                mo_ps = mps.tile([128, HID], F32, tag="mo_ps")
                for ic in range(NIC):
                    for jh in range(NJH):
                        nc.tensor.matmul(
                            mo_ps[:, jh * 512:(jh + 1) * 512],
                            atT_sb[:, ic, :],
                            wo_sb[:, ic, jh * 512:(jh + 1) * 512],
                            start=(ic == 0), stop=(ic == NIC - 1),
                        )

                # gate ao-branch: ao @ (Wo @ Wg2) folded on host (bf16: fp8
                # noise here spikes the max-err tail, and the fold already
                # removed the transpose chain)
                for ic in range(NIC):
                    for jh in range(NJH):
                        sl = slice(jh * 512, (jh + 1) * 512)
                        nc.tensor.matmul(
                            g_ps[:, sl], atT_sb[:, ic, :],
                            wog_sb[:, ic, sl],
                            start=False, stop=(ic == NIC - 1))

                mo_sb = csb.tile([128, HID], F32, tag="mo")
                nc.scalar.copy(mo_sb[:], mo_ps[:])

                gb_sb = csb.tile([128, HID], F32, tag="gb")
                nc.vector.tensor_add(gb_sb[:], g_ps[:], bgb_sb[:])
                # sigmoid(x) = 0.5*tanh(x/2) + 0.5 (tanh shares ACT set w/ exp)
                # scale folds out the host Wg pre-scale
                nc.scalar.activation(gb_sb[:], gb_sb[:], AF.Tanh,
                                     scale=0.5 / WS)

                # aug = h + g*mo = (h + 0.5*mo) + (0.5*mo)*tanh
                u_sb = csb.tile([128, HID], F32, tag="u")
                nc.vector.scalar_tensor_tensor(
                    u_sb[:], mo_sb[:], 0.5, h_sb[:], op0=OP.mult, op1=OP.add)
                v_sb = csb.tile([128, HID], F32, tag="v")
                nc.vector.scalar_tensor_tensor(
                    v_sb[:], gb_sb[:], 0.5, mo_sb[:], op0=OP.mult, op1=OP.mult)
                nc.vector.scalar_tensor_tensor(
                    u_sb[:], u_sb[:], 0.0, v_sb[:], op0=OP.add, op1=OP.add,
                    accum_out=sum_all[:, t:t + 1])
                # square's tensor output is scrap; we only keep the accumulator
                nc.scalar.activation(
                    v_sb[:], u_sb[:], AF.Square, accum_out=ss_all[:, t:t + 1])

                # ---- LayerNorm finalize, per tile, VectorE only ----
                mean = stp.tile([128, 1], F32, tag="mean")
                nc.vector.tensor_scalar_mul(mean[:], sum_all[:, t:t + 1], 1.0 / HID)
                m2 = stp.tile([128, 1], F32, tag="m2")
                nc.vector.tensor_mul(m2[:], mean[:], mean[:])
                nc.vector.tensor_scalar_add(m2[:], m2[:], -LN_EPS)
                vpe = stp.tile([128, 1], F32, tag="vpe")
                nc.vector.scalar_tensor_tensor(
                    vpe[:], ss_all[:, t:t + 1], 1.0 / HID, m2[:],
                    op0=OP.mult, op1=OP.subtract)
                # rstd = 1/sqrt(vpe): quake init + 3 Newton iterations
                y = stp.tile([128, 1], F32, tag="y")
                yi = y[:].bitcast(I32)
                nc.vector.tensor_scalar(
                    yi, vpe[:].bitcast(I32), 1, None,
                    op0=OP.logical_shift_right)
                nc.vector.tensor_scalar(
                    yi, yi, -RSQRT_MAGIC, -1,
                    op0=OP.add, op1=OP.mult)
                yy = stp.tile([128, 1], F32, tag="yy")
                hw = stp.tile([128, 1], F32, tag="hw")
                for _ in range(3):
                    nc.vector.tensor_mul(yy[:], y[:], y[:])
                    nc.vector.tensor_mul(yy[:], yy[:], vpe[:])
                    nc.vector.tensor_scalar(
                        hw[:], yy[:], -0.5, 1.5, op0=OP.mult, op1=OP.add)
                    nc.vector.tensor_mul(y[:], y[:], hw[:])

                # yout = (aug - mean)*rstd*lng + lnb
                nc.vector.scalar_tensor_tensor(
                    u_sb[:], u_sb[:], mean[:], lng_sb[:],
                    op0=OP.subtract, op1=OP.mult)
                yo_sb = csb.tile([128, HID], F32, tag="yo")
                nc.vector.scalar_tensor_tensor(
                    yo_sb[:], u_sb[:], y[:], lnb_sb[:],
                    op0=OP.mult, op1=OP.add)
                nc.sync.dma_start(out_d.ap()[t], yo_sb[:])

        pHT_cm.__exit__(None, None, None)   # release hT
        pAO_cm.__exit__(None, None, None)   # release attnout
        pWO_cm.__exit__(None, None, None)   # release Wo/Wg/Wog/eye

    nc.compile()
    return nc


def _prep_core(hs, mk, mv, nt):
    """Host-side lossless layout prep for one core's shard."""
    hT = np.ascontiguousarray(
        hs.reshape(nt, 128, NIC, 128).transpose(0, 3, 2, 1))      # [t,p,ic,b]
    h = np.ascontiguousarray(hs.reshape(nt, 128, HID))
    mkT = np.ascontiguousarray(
        mk.reshape(nt, 128, TOPK, NIC, 128).transpose(0, 2, 4, 3, 1))
    mvT = np.ascontiguousarray(
        mv.reshape(nt, 128, TOPK, NIC, 128).transpose(0, 2, 4, 3, 1))
    return hT, h, mkT, mvT


def kernel(**inputs):
    hs = np.asarray(inputs["hidden_state"], dtype=np.float32)
    mk = np.asarray(inputs["memory_keys"], dtype=np.float32)
    mv = np.asarray(inputs["memory_values"], dtype=np.float32)

    import ml_dtypes
    bf = ml_dtypes.bfloat16
    f8 = ml_dtypes.float8_e4m3

    def wlay(w):
        return np.ascontiguousarray(
            np.asarray(w, np.float32).reshape(NIC, 128, HID).transpose(1, 0, 2))

    wg_full = np.asarray(inputs["Wg"], np.float32)
    wo_f32 = np.asarray(inputs["Wo"], np.float32)
    wog_f32 = wo_f32 @ wg_full[HID:]          # fold Wo into the gate branch

    wq = (wlay(inputs["Wq"]) * WS).astype(f8)
    wk = (wlay(inputs["Wk"]) * WS).astype(f8)
    wv = wlay(inputs["Wv"]).astype(bf)
    wo = wlay(wo_f32).astype(bf)
    wg1 = (wlay(wg_full[:HID]) * WS).astype(f8)
    wog = (wlay(wog_f32) * WS).astype(bf)
    bgb = np.ascontiguousarray(
        np.broadcast_to(np.asarray(inputs["bg"], np.float32) * WS, (128, HID)))
    lng = np.ascontiguousarray(
        np.broadcast_to(np.asarray(inputs["ln_g"], np.float32), (128, HID)))
    lnb = np.ascontiguousarray(
        np.broadcast_to(np.asarray(inputs["ln_b"], np.float32), (128, HID)))
    eye = np.eye(128, dtype=np.float32)

    if "nc" not in _CACHE:
        _CACHE["nc"] = _build(NT)
    nc = _CACHE["nc"]

    in_maps = []
    for c in range(N_CORES):
        sl = slice(c * BC, (c + 1) * BC)
        hT, h, mkT, mvT = _prep_core(hs[sl], mk[sl], mv[sl], NT)
        in_maps.append({
            "hT8": hT.astype(f8), "h": h,
            "mkT": mkT.astype(f8), "mvT": mvT.astype(bf),
            "Wq": wq, "Wk": wk, "Wv": wv, "Wo": wo, "Wg1": wg1, "Wog": wog,
            "bgB": bgb, "eye": eye, "lngB": lng, "lnbB": lnb,
        })

    res = run_bass_kernel_spmd(nc, in_maps, core_ids=list(range(N_CORES)),
                               trace=TRACE)
    kernel.last_result = res
    out = np.concatenate(
        [r["out"].reshape(BC, HID) for r in res.results], axis=0)
    return out


kernel.last_result = None


# revision 44
# speedup vs baseline: 1.3845x; 1.0747x over previous
"""Trainium2 Bass kernel for nn_EngramModule: single-query top-k memory attention
with gated residual + LayerNorm, data-parallel across 8 NeuronCores.

Contract: kernel(**inputs) takes the FULL unsharded inputs and returns the FULL
(8192, 1024) float32 output.

Per-core pipeline (1024 batch rows, 8 row-tiles of 128):
  A+B (merged, software-pipelined per tile): Q = h @ Wq in bf16, then per
      k-slot: Kp projection in fp8 e4m3 DoubleRow mode (256-row contraction
      per instruction, ~2x bf16 throughput; Wk pre-scaled by 32 on host to
      dodge e4m3 subnormals, folded back in the exp scale); Vp projection in
      bf16 (error budget: V feeds the output directly, the K path only
      perturbs softmax weights).  Kp is evicted PSUM->SBUF as bf16 by
      ScalarE so the score multiply+reduce run the DVE 2x packed mode;
      attn-weighted V accumulates in f32 with GpSimd adds (its only job).
  C:  memory_out = attnout @ Wo in bf16.  The gate's mo-branch is folded on
      host: mo @ Wg2 = ao @ (Wo @ Wg2) = ao @ Wog, so no mo transpose or
      second eviction chain exists; gate = hT8 @ Wg1 + aoT8 @ Wog, both in
      fp8 DoubleRow (weights *32, folded into the tanh scale).
      sigmoid(x) = 0.5*tanh(x/2)+0.5 keeps ScalarE in one ACT table set;
      aug = h + g*mo; LayerNorm finalizes per tile with a VectorE-only
      Newton rsqrt (bit-trick init).

Bulk weight streams ride the ScalarE HWDGE queue; latency-critical per-tile
loads ride the SyncE queue so they are never stuck behind megabyte weight
transfers. Activations are pre-laid-out on host (pure transpose/reshape,
lossless) so the contraction dim sits on SBUF partitions and no on-chip
transposes of the big tensors are needed.
"""

import os
import sys

import numpy as np

for _p in ("/opt/trn_rl_repo", "/root/.axon_site/_ro/trn_rl_repo"):
    if os.path.isdir(_p) and _p not in sys.path:
        sys.path.insert(0, _p)

from contextlib import ExitStack

import concourse.bacc as bacc
import concourse.mybir as mybir
import concourse.tile as tile
from concourse.bass_utils import run_bass_kernel_spmd

F32 = mybir.dt.float32
F32R = mybir.dt.float32r
BF16 = mybir.dt.bfloat16
F8 = mybir.dt.float8e4
I32 = mybir.dt.int32
AX = mybir.AxisListType
OP = mybir.AluOpType
AF = mybir.ActivationFunctionType
DR = mybir.MatmulPerfMode.DoubleRow

N_CORES = 8
B = 8192
HID = 1024
NH = 16
DH = 64
TOPK = 8
LN_EPS = 1e-5

BC = B // N_CORES          # rows per core = 1024
NT = BC // 128             # row-tiles per core = 8
NIC = HID // 128           # 128-row contraction chunks = 8
NICP = NIC // 2            # fp8 DoubleRow 256-row chunk pairs = 4
NJH = HID // 512           # 512-wide output halves = 2
SCALE = DH ** -0.5
WS = 32.0                  # host pre-scale on fp8 weights (power of 2)
RSQRT_MAGIC = 0x5F3759DF

# Set by test.py to collect a profile; grading path leaves this off.
TRACE = False

_CACHE = {}


def _build(nt=NT):
    nc = bacc.Bacc("TRN2", target_bir_lowering=False, debug=False,
                   num_devices=N_CORES)

    # ---- DRAM parameters (per-core shard, host-prepped layouts) ----
    h_d = nc.declare_dram_parameter("h", [nt, 128, HID], F32, isOutput=False)
    mkT_d = nc.declare_dram_parameter("mkT", [nt, TOPK, 128, NIC, 128], F8, isOutput=False)
    mvT8_d = nc.declare_dram_parameter("mvT8", [nt, TOPK, 128, NICP, 128], F8, isOutput=False)
    mvTb_d = nc.declare_dram_parameter("mvTb", [nt, TOPK, 128, NICP, 128], BF16, isOutput=False)
    hTb_d = nc.declare_dram_parameter("hTb", [nt, 128, NIC, 128], BF16, isOutput=False)
    hT8_d = nc.declare_dram_parameter("hT8", [nt, 128, NIC, 128], F8, isOutput=False)
    wq_d = nc.declare_dram_parameter("Wq", [128, NIC, HID], BF16, isOutput=False)
    wk_d = nc.declare_dram_parameter("Wk", [128, NIC, HID], F8, isOutput=False)
    wv8_d = nc.declare_dram_parameter("Wv8", [128, NICP, HID], F8, isOutput=False)
    wvb_d = nc.declare_dram_parameter("Wvb", [128, NICP, HID], BF16, isOutput=False)
    wo_d = nc.declare_dram_parameter("Wo", [128, NIC, HID], BF16, isOutput=False)
    wg1_d = nc.declare_dram_parameter("Wg1", [128, NIC, HID], F8, isOutput=False)
    wog_d = nc.declare_dram_parameter("Wog", [128, NIC, HID], BF16, isOutput=False)
    bgb_d = nc.declare_dram_parameter("bgB", [128, HID], F32, isOutput=False)
    eye_d = nc.declare_dram_parameter("eye", [128, 128], F32, isOutput=False)
    lng_d = nc.declare_dram_parameter("lngB", [128, HID], F32, isOutput=False)
    lnb_d = nc.declare_dram_parameter("lnbB", [128, HID], F32, isOutput=False)
    out_d = nc.declare_dram_parameter("out", [nt, 128, HID], F32, isOutput=True)

    def load_w(tile_sb, dram, nchunk):
        # bulk weights on the ScalarE HWDGE queue, chunked so the first
        # dependent matmul only waits for its own chunk
        for ic in range(nchunk):
            nc.scalar.dma_start(tile_sb[:, ic, :], dram.ap()[:, ic, :])

    with ExitStack() as octx:
        tc = octx.enter_context(tile.TileContext(nc))

        pers = octx.enter_context(tc.tile_pool(name="pers", bufs=1))
        sum_all = pers.tile([128, nt], F32, tag="sum_all")
        ss_all = pers.tile([128, nt], F32, tag="ss_all")

        # long-lived weights / cross-phase activations
        pWO_cm = tc.tile_pool(name="pWO", bufs=1); pWO = pWO_cm.__enter__()
        wo_sb = pWO.tile([128, NIC, HID], BF16, tag="wo")
        wg1_sb = pWO.tile([128, NIC, HID], F8, tag="wg1")
        wog_sb = pWO.tile([128, NIC, HID], BF16, tag="wog")
        eye_sb = pWO.tile([128, 128], F32, tag="eye")

        pAO_cm = tc.tile_pool(name="pAO", bufs=1); pAO = pAO_cm.__enter__()
        ao_all = pAO.tile([128, nt, HID], F32, tag="ao_all")

        # resident fp8 hidden-state transpose for the phase-C gate h-branch
        pHT_cm = tc.tile_pool(name="pHT", bufs=1); pHT = pHT_cm.__enter__()
        hT8_all = pHT.tile([128, nt, NIC, 128], F8, tag="hT8")

        pAB_cm = tc.tile_pool(name="pAB", bufs=1); pAB = pAB_cm.__enter__()
        q_all = pAB.tile([128, nt, HID], BF16, tag="q_all")

        pWKV_cm = tc.tile_pool(name="wkv", bufs=1); wkv = pWKV_cm.__enter__()
        wq_sb = wkv.tile([128, NIC, HID], BF16, tag="wq")
        wk_sb = wkv.tile([128, NIC, HID], F8, tag="wk")
        wv8_sb = wkv.tile([128, NICP, HID], F8, tag="wv8")
        wvb_sb = wkv.tile([128, NICP, HID], BF16, tag="wvb")

        # ============ merged phase A+B: Q projection + attention ============
        with ExitStack() as bctx:
            hp = bctx.enter_context(tc.tile_pool(name="hT_a", bufs=1))
            mp = bctx.enter_context(tc.tile_pool(name="mkv", bufs=4))
            kvps = bctx.enter_context(tc.tile_pool(name="kv_ps", bufs=2, space="PSUM"))
            sp = bctx.enter_context(tc.tile_pool(name="scr", bufs=2))
            accp = bctx.enter_context(tc.tile_pool(name="acc", bufs=2))
            ep = bctx.enter_context(tc.tile_pool(name="e", bufs=2))

            # startup loads fan out over all three DMA queues so the first
            # tile isn't serialized behind one queue: scalar takes wq/wk/wv
            # (chunked, consumed first), gpsimd (software DGE, otherwise
            # idle) takes everything phase C needs, sync takes hT plus the
            # mkT/mvT stream.
            load_w(wq_sb, wq_d, NIC)
            load_w(wk_sb, wk_d, NIC)
            load_w(wv8_sb, wv8_d, NICP)
            load_w(wvb_sb, wvb_d, NICP)
            nc.gpsimd.dma_start(wo_sb[:], wo_d.ap())
            nc.gpsimd.dma_start(wg1_sb[:], wg1_d.ap())
            nc.gpsimd.dma_start(wog_sb[:], wog_d.ap())
            nc.gpsimd.dma_start(eye_sb[:], eye_d.ap())
            for t in range(nt):
                nc.gpsimd.dma_start(hT8_all[:, t], hT8_d.ap()[t])

            # resident hT (bf16) for the Q projection, ahead of the mk/mv
            # stream on the sync queue
            hT_all = hp.tile([128, nt, NIC, 128], BF16, tag="hT")
            for t in range(nt):
                nc.sync.dma_start(hT_all[:, t], hTb_d.ap()[t])
            def load_slot(t, k):
                a = mp.tile([128, NIC, 128], F8, tag="mkT")
                nc.sync.dma_start(a[:], mkT_d.ap()[t, k])
                b8 = mp.tile([128, NICP, 128], F8, tag="mvT8")
                nc.sync.dma_start(b8[:], mvT8_d.ap()[t, k])
                bb = mp.tile([128, NICP, 128], BF16, tag="mvTb")
                nc.sync.dma_start(bb[:], mvTb_d.ap()[t, k])
                return (a, b8, bb)

            preload = {(0, 0): load_slot(0, 0)}

            for t in range(nt):
                # ---- Q projection for this tile (PSUM slot shared w/ kp) ----
                q_ps = kvps.tile([128, HID], F32, tag="kp")
                for ic in range(NIC):
                    for jh in range(NJH):
                        nc.tensor.matmul(
                            q_ps[:, jh * 512:(jh + 1) * 512],
                            hT_all[:, t, ic, :],
                            wq_sb[:, ic, jh * 512:(jh + 1) * 512],
                            start=(ic == 0), stop=(ic == NIC - 1),
                        )
                nc.scalar.copy(q_all[:, t, :], q_ps[:])

                acc = accp.tile([128, HID], BF16, tag="acc")
                e_all = ep.tile([128, TOPK, NH], BF16, tag="e_all")
                q_t = q_all[:, t, :]
                for k in range(TOPK):
                    if (t, k) in preload:
                        mkT, mvT8, mvTb = preload[(t, k)]
                    else:
                        mkT, mvT8, mvTb = load_slot(t, k)

                    kp_ps = kvps.tile([128, HID], F32, tag="kp")
                    vp_ps = kvps.tile([128, HID], F32, tag="vp")
                    # K projection: fp8 DoubleRow, 256-row contraction pairs
                    for ip in range(NICP):
                        for jh in range(NJH):
                            nc.tensor.matmul(
                                kp_ps[:, jh * 512:(jh + 1) * 512],
                                mkT[:, 2 * ip:2 * ip + 2, :],
                                wk_sb[:, 2 * ip:2 * ip + 2, jh * 512:(jh + 1) * 512],
                                start=(ip == 0), stop=(ip == NICP - 1),
                                perf_mode=DR,
                            )
                    # V projection, split precision: contraction rows 0..511
                    # in fp8 DoubleRow, rows 512..1023 in bf16 (V feeds the
                    # output directly, so only half rides fp8)
                    for ip in range(NICP // 2):
                        for jh in range(NJH):
                            nc.tensor.matmul(
                                vp_ps[:, jh * 512:(jh + 1) * 512],
                                mvT8[:, 2 * ip:2 * ip + 2, :],
                                wv8_sb[:, 2 * ip:2 * ip + 2, jh * 512:(jh + 1) * 512],
                                start=(ip == 0), stop=False,
                                perf_mode=DR,
                            )
                    for ic in range(NICP):
                        for jh in range(NJH):
                            nc.tensor.matmul(
                                vp_ps[:, jh * 512:(jh + 1) * 512],
                                mvTb[:, ic, :],
                                wvb_sb[:, ic, jh * 512:(jh + 1) * 512],
                                start=False, stop=(ic == NICP - 1),
                            )

                    # scores for all 16 heads of this k-slot (bf16 products
                    # so the per-head reduce reads packed 16-bit)
                    p_scr = sp.tile([128, HID], BF16, tag="p")
                    nc.vector.tensor_mul(p_scr[:], q_t, kp_ps[:])
                    s_k = ep.tile([128, NH], F32, tag="s_k")
                    nc.vector.reduce_sum(
                        s_k[:], p_scr[:].rearrange("p (h d) -> p h d", h=NH), axis=AX.X)
                    # e = exp(scores * DH**-0.5 / WS); logits ~N(0,1) so no
                    # max-sub; /WS folds out the host Wk pre-scale
                    nc.scalar.activation(e_all[:, k, :], s_k[:], AF.Exp,
                                         scale=SCALE / WS)

                    # weighted V accumulate: acc += e[:,k,h] (bcast over d) * Vp
                    e_bc = e_all[:, k, :].unsqueeze(2).broadcast_to([128, NH, DH])
                    dst = acc if k == 0 else sp.tile([128, HID], BF16, tag="pv")
                    nc.vector.tensor_tensor(
                        dst[:].rearrange("p (h d) -> p h d", h=NH),
                        vp_ps[:].rearrange("p (h d) -> p h d", h=NH),
                        e_bc, op=OP.mult)
                    if k > 0:
                        nc.vector.tensor_add(acc[:], acc[:], dst[:])

                # normalize: attnout = acc * (1/(WS * sum_k e)); the extra WS
                # folds out the host Wv pre-scale riding on acc
                den = ep.tile([128, NH], F32, tag="den")
                nc.vector.reduce_sum(
                    den[:], e_all[:].rearrange("p k h -> p h k"), axis=AX.X)
                rden = ep.tile([128, NH], F32, tag="rden")
                nc.vector.tensor_scalar_mul(den[:], den[:], WS)
                nc.vector.reciprocal(rden[:], den[:])
                rden_bc = rden[:].unsqueeze(2).broadcast_to([128, NH, DH])
                nc.vector.tensor_tensor(
                    ao_all[:, t, :].rearrange("p (h d) -> p h d", h=NH),
                    acc[:].rearrange("p (h d) -> p h d", h=NH),
                    rden_bc, op=OP.mult)

        pWKV_cm.__exit__(None, None, None)  # release Wq/Wk/Wv
        pAB_cm.__exit__(None, None, None)   # release q_all

        # ===== phase C: memory_out, gate, residual, LayerNorm, output =====
        with ExitStack() as cctx:
            eyep = cctx.enter_context(tc.tile_pool(name="cconst", bufs=1))
            cstr = cctx.enter_context(tc.tile_pool(name="c_str", bufs=2))
            csb = cctx.enter_context(tc.tile_pool(name="c_sb", bufs=2))
            stp = cctx.enter_context(tc.tile_pool(name="stats", bufs=2))
            tps = cctx.enter_context(tc.tile_pool(name="tp_ps", bufs=1, space="PSUM"))
            mps = cctx.enter_context(tc.tile_pool(name="mo_ps", bufs=1, space="PSUM"))
            gps = cctx.enter_context(tc.tile_pool(name="g_ps", bufs=2, space="PSUM"))

            # epilogue constants on the GpSimd queue (idle during B/C)
            bgb_sb = eyep.tile([128, HID], F32, tag="bgb")
            nc.gpsimd.dma_start(bgb_sb[:], bgb_d.ap())
            lng_sb = eyep.tile([128, HID], F32, tag="lng")
            nc.gpsimd.dma_start(lng_sb[:], lng_d.ap())
            lnb_sb = eyep.tile([128, HID], F32, tag="lnb")
            nc.gpsimd.dma_start(lnb_sb[:], lnb_d.ap())

            for t in range(nt):
                h_sb = cstr.tile([128, HID], F32, tag="h_c")
                nc.gpsimd.dma_start(h_sb[:], h_d.ap()[t])

                # attn-out transposes, then both evictions (bf16 for Wo,
                # fp8 for the gate's Wog branch)
                at_ps = tps.tile([128, NIC, 128], F32, tag="tp_ps")
                for ic in range(NIC):
                    nc.tensor.transpose(
                        at_ps[:, ic, :], ao_all[:, t, ic * 128:(ic + 1) * 128],
                        eye_sb[:])
                atT_sb = csb.tile([128, NIC, 128], BF16, tag="atT")
                nc.scalar.copy(atT_sb[:], at_ps[:])

                # gate h-branch: fp8 DoubleRow; fills TensorE while the
                # evictions drain on ScalarE
                g_ps = gps.tile([128, HID], F32, tag="g_ps")
                for ip in range(NICP):
                    for jh in range(NJH):
                        sl = slice(jh * 512, (jh + 1) * 512)
                        nc.tensor.matmul(
                            g_ps[:, sl], hT8_all[:, t, 2 * ip:2 * ip + 2, :],
                            wg1_sb[:, 2 * ip:2 * ip + 2, sl],
                            start=(ip == 0), stop=False, perf_mode=DR)

                mo_ps = mps.tile([128, HID], F32, tag="mo_ps")
                for ic in range(NIC):
                    for jh in range(NJH):
                        nc.tensor.matmul(
                            mo_ps[:, jh * 512:(jh + 1) * 512],
                            atT_sb[:, ic, :],
                            wo_sb[:, ic, jh * 512:(jh + 1) * 512],
                            start=(ic == 0), stop=(ic == NIC - 1),
                        )

                # gate ao-branch: ao @ (Wo @ Wg2) folded on host (bf16: fp8
                # noise here spikes the max-err tail, and the fold already
                # removed the transpose chain)
                for ic in range(NIC):
                    for jh in range(NJH):
                        sl = slice(jh * 512, (jh + 1) * 512)
                        nc.tensor.matmul(
                            g_ps[:, sl], atT_sb[:, ic, :],
                            wog_sb[:, ic, sl],
                            start=False, stop=(ic == NIC - 1))

                mo_sb = csb.tile([128, HID], F32, tag="mo")
                nc.scalar.copy(mo_sb[:], mo_ps[:])

                gb_sb = csb.tile([128, HID], F32, tag="gb")
                nc.vector.tensor_add(gb_sb[:], g_ps[:], bgb_sb[:])
                # sigmoid(x) = 0.5*tanh(x/2) + 0.5 (tanh shares ACT set w/ exp)
                # scale folds out the host Wg pre-scale
                nc.scalar.activation(gb_sb[:], gb_sb[:], AF.Tanh,
                                     scale=0.5 / WS)

                # aug = h + g*mo = (h + 0.5*mo) + (0.5*mo)*tanh
                u_sb = csb.tile([128, HID], F32, tag="u")
                nc.vector.scalar_tensor_tensor(
                    u_sb[:], mo_sb[:], 0.5, h_sb[:], op0=OP.mult, op1=OP.add)
                v_sb = csb.tile([128, HID], F32, tag="v")
                nc.vector.scalar_tensor_tensor(
                    v_sb[:], gb_sb[:], 0.5, mo_sb[:], op0=OP.mult, op1=OP.mult)
                nc.vector.scalar_tensor_tensor(
                    u_sb[:], u_sb[:], 0.0, v_sb[:], op0=OP.add, op1=OP.add,
                    accum_out=sum_all[:, t:t + 1])
                # square's tensor output is scrap; we only keep the accumulator
                nc.scalar.activation(
                    v_sb[:], u_sb[:], AF.Square, accum_out=ss_all[:, t:t + 1])

                # ---- LayerNorm finalize, per tile, VectorE only ----
                mean = stp.tile([128, 1], F32, tag="mean")
                nc.vector.tensor_scalar_mul(mean[:], sum_all[:, t:t + 1], 1.0 / HID)
                m2 = stp.tile([128, 1], F32, tag="m2")
                nc.vector.tensor_mul(m2[:], mean[:], mean[:])
                nc.vector.tensor_scalar_add(m2[:], m2[:], -LN_EPS)
                vpe = stp.tile([128, 1], F32, tag="vpe")
                nc.vector.scalar_tensor_tensor(
                    vpe[:], ss_all[:, t:t + 1], 1.0 / HID, m2[:],
                    op0=OP.mult, op1=OP.subtract)
                # rstd = 1/sqrt(vpe): quake init + 3 Newton iterations
                y = stp.tile([128, 1], F32, tag="y")
                yi = y[:].bitcast(I32)
                nc.vector.tensor_scalar(
                    yi, vpe[:].bitcast(I32), 1, None,
                    op0=OP.logical_shift_right)
                nc.vector.tensor_scalar(
                    yi, yi, -RSQRT_MAGIC, -1,
                    op0=OP.add, op1=OP.mult)
                yy = stp.tile([128, 1], F32, tag="yy")
                hw = stp.tile([128, 1], F32, tag="hw")
                for _ in range(3):
                    nc.vector.tensor_mul(yy[:], y[:], y[:])
                    nc.vector.tensor_mul(yy[:], yy[:], vpe[:])
                    nc.vector.tensor_scalar(
                        hw[:], yy[:], -0.5, 1.5, op0=OP.mult, op1=OP.add)
                    nc.vector.tensor_mul(y[:], y[:], hw[:])

                # yout = (aug - mean)*rstd*lng + lnb
                nc.vector.scalar_tensor_tensor(
                    u_sb[:], u_sb[:], mean[:], lng_sb[:],
                    op0=OP.subtract, op1=OP.mult)
                yo_sb = csb.tile([128, HID], F32, tag="yo")
                nc.vector.scalar_tensor_tensor(
                    yo_sb[:], u_sb[:], y[:], lnb_sb[:],
                    op0=OP.mult, op1=OP.add)
                nc.sync.dma_start(out_d.ap()[t], yo_sb[:])

        pHT_cm.__exit__(None, None, None)   # release hT
        pAO_cm.__exit__(None, None, None)   # release attnout
        pWO_cm.__exit__(None, None, None)   # release Wo/Wg/Wog/eye

    nc.compile()
    return nc


def _prep_core(hs, mk, mv, nt):
    """Host-side lossless layout prep for one core's shard."""
    hT = np.ascontiguousarray(
        hs.reshape(nt, 128, NIC, 128).transpose(0, 3, 2, 1))      # [t,p,ic,b]
    h = np.ascontiguousarray(hs.reshape(nt, 128, HID))
    mkT = np.ascontiguousarray(
        mk.reshape(nt, 128, TOPK, NIC, 128).transpose(0, 2, 4, 3, 1))
    mvT = np.ascontiguousarray(
        mv.reshape(nt, 128, TOPK, NIC, 128).transpose(0, 2, 4, 3, 1))
    return hT, h, mkT, mvT


def kernel(**inputs):
    hs = np.asarray(inputs["hidden_state"], dtype=np.float32)
    mk = np.asarray(inputs["memory_keys"], dtype=np.float32)
    mv = np.asarray(inputs["memory_values"], dtype=np.float32)

    import ml_dtypes
    bf = ml_dtypes.bfloat16
    f8 = ml_dtypes.float8_e4m3

    def wlay(w):
        return np.ascontiguousarray(
            np.asarray(w, np.float32).reshape(NIC, 128, HID).transpose(1, 0, 2))

    wg_full = np.asarray(inputs["Wg"], np.float32)
    wo_f32 = np.asarray(inputs["Wo"], np.float32)
    wog_f32 = wo_f32 @ wg_full[HID:]          # fold Wo into the gate branch

    wq = wlay(inputs["Wq"]).astype(bf)
    wk = (wlay(inputs["Wk"]) * WS).astype(f8)
    wv_s = wlay(inputs["Wv"]) * WS
    wv8 = np.ascontiguousarray(wv_s[:, :NICP]).astype(f8)
    wvb = np.ascontiguousarray(wv_s[:, NICP:]).astype(bf)
    wo = wlay(wo_f32).astype(bf)
    wg1 = (wlay(wg_full[:HID]) * WS).astype(f8)
    wog = (wlay(wog_f32) * WS).astype(bf)
    bgb = np.ascontiguousarray(
        np.broadcast_to(np.asarray(inputs["bg"], np.float32) * WS, (128, HID)))
    lng = np.ascontiguousarray(
        np.broadcast_to(np.asarray(inputs["ln_g"], np.float32), (128, HID)))
    lnb = np.ascontiguousarray(
        np.broadcast_to(np.asarray(inputs["ln_b"], np.float32), (128, HID)))
    eye = np.eye(128, dtype=np.float32)

    if "nc" not in _CACHE:
        _CACHE["nc"] = _build(NT)
    nc = _CACHE["nc"]

    in_maps = []
    for c in range(N_CORES):
        sl = slice(c * BC, (c + 1) * BC)
        hT, h, mkT, mvT = _prep_core(hs[sl], mk[sl], mv[sl], NT)
        in_maps.append({
            "hTb": hT.astype(bf), "hT8": hT.astype(f8), "h": h,
            "mkT": mkT.astype(f8),
            "mvT8": np.ascontiguousarray(mvT[:, :, :, :NICP]).astype(f8),
            "mvTb": np.ascontiguousarray(mvT[:, :, :, NICP:]).astype(bf),
            "Wq": wq, "Wk": wk, "Wv8": wv8, "Wvb": wvb, "Wo": wo,
            "Wg1": wg1, "Wog": wog,
            "bgB": bgb, "eye": eye, "lngB": lng, "lnbB": lnb,
        })

    res = run_bass_kernel_spmd(nc, in_maps, core_ids=list(range(N_CORES)),
                               trace=TRACE)
    kernel.last_result = res
    out = np.concatenate(
        [r["out"].reshape(BC, HID) for r in res.results], axis=0)
    return out


kernel.last_result = None
